# revision 36
# baseline (speedup 1.0000x reference)
"""TRN2 Bass kernel for causal multi-head attention with RoPE.

Problem: B=2, S=2048, HID=2048, NH=16, HD=128 (fp32 in/out).
Sharding: 8 cores = 2 (batch) x 4 (head-groups of 4 heads).
Each core computes q/k/v projections for its 4 heads (column-parallel),
RoPE, causal attention, and a row-parallel partial o_proj; the host sums
the 4 partials per batch.

v2 (363us -> ~287us): fp8 (e4m3) DoubleRow matmuls at 2x bf16 PE rate
for the bulk of the work, exploiting the loose 2e-2 rel-err gate
(final rel err ~8.5e-3). Error analysis: softmax here is broad (logit
std ~= 1), so iid fp8 quantization noise on q/k/v/P averages down by
~1/sqrt(N_keys) for late queries; only EARLY queries (few keys) and
the final o_proj see fp8 noise unattenuated. Hence:
  - queries/keys 0..255 run a bf16/fp16 exact path (bf16 q/k/v
    projections, fp16 probabilities, fp16 V copy),
  - everything else uses fp8 DoubleRow for Q/K/V projections and P@V
    in pairs of adjacent key tiles (P stored fp8 with exp bias -2 so
    values fit e4m3's +-240 range - the bias cancels in softmax
    normalization),
  - scores stay bf16: DoubleRow needs the d=128 contraction split to
    K=64, and K=64 DR measures HALF the K=128 rate (417 vs 211 ns per
    512-col matmul) - zero gain, so don't (tried, reverted),
  - o_proj stays bf16 (no averaging after it; fp8 would be ~3.5% err).
Weights are host-prescaled by 32 (W elems ~ N(0, 1/2048) would land in
e4m3's subnormal range); the 1/32 is folded into the RoPE tables (bf16)
and the V-eviction scale.

Engine-placement lessons (measured, not guessed):
  - GpSimd DSP is ~3x slower than DVE per tensor op (1.15us vs ~0.35us
    for a [128,512] fp16 add) and swapping op kinds forces ~7us library
    reloads; it now does ONLY dma_start issue. (Moving rope-adds or
    smacc there cost 50-290us total - reverted.)
  - Softmax sums accumulate on the PE: per-pair ones8 [128,2,128]
    DoubleRow matmuls into a psum whose 128 identical rows make the
    sum pre-broadcast, so the normalize chain is just reciprocal (DVE,
    full height) -> aot multiply. No partition_broadcast at all.
    (DVE smacc adds convoy the mask->PV chain: +80us - reverted.)
  - fp8 stale-byte hazard: pt8 ring slots hold old fp16 bytes that
    alias to e4m3 NaN, and NaN*0=NaN, so diagonal-pair gaps are
    memset to 0 instead of relying on the mask multiply.
  - Phase P runs fp8-first: the first V matmuls gate on ~0.5MB of fp8
    stream instead of 2.6MB of bf16 under the ~20us DMA bandwidth
    ramp; bf16 streams (xc0, wv, wq/wk) land during fp8 compute.
  - Q and K are projected per head, head 0 first, so attention starts
    while later heads' RoPE evictions drain the DVE queue.
  - Do NOT emit chunk 2's o_proj inside chunk 3's head loop: +55us
    (tried, reverted; cause unclear - keep emission after the loop).

Carried over from v1: SBUF-resident per-head QT/KT, 4-deep weight tile
rings, staged per-head normalize chain (one stage pumped per tile of
the following head), o_proj deferred one chunk and its bf16 partials
spread over 3 DMA queues.
"""
import os
import sys

if "/opt/trn_rl_repo" not in sys.path:
    sys.path.insert(0, "/opt/trn_rl_repo")

import numpy as np
import ml_dtypes

import concourse.bass as bass
import concourse.mybir as mybir
import concourse.tile as tile
from concourse import bacc
from concourse.bass_utils import run_bass_kernel_spmd
from contextlib import ExitStack

P = 128
B, S, HID, NH = 2, 2048, 2048, 16
HD = HID // NH              # 128
H = 4                       # heads per core
DPC = H * HD                # 512 dims per core
KO = HID // P               # 16 contraction chunks
SC = S // 512               # 4 seq chunks of 512
ST = S // P                 # 16 seq tiles of 128
SCALE = 1.0 / float(np.sqrt(HD))
WS = 32.0                   # host weight prescale (fp8 range)
PB = -2.0                   # exp bias for fp8 P (cancels in softmax)

f32 = mybir.dt.float32
bf16 = mybir.dt.bfloat16
fp16 = mybir.dt.float16
fp8 = mybir.dt.float8e4
DR = mybir.MatmulPerfMode.DoubleRow

_CACHED_NC = None


def build_nc():
    DBG = os.environ.get("KDBG", "0") == "1"
    AF = mybir.ActivationFunctionType
    nc = bacc.Bacc(None, target_bir_lowering=False)

    # chunk 0 of x in bf16 (early-query exact path); chunks 1-3 fp8
    xt = nc.declare_dram_parameter("xt", [P, KO, 256], bf16, isOutput=False)
    xt8 = nc.declare_dram_parameter("xt8", [P, KO, 1792], fp8, isOutput=False)
    wq = nc.declare_dram_parameter("wq", [H, P, KO, HD], bf16, isOutput=False)
    wk = nc.declare_dram_parameter("wk", [H, P, KO, HD], bf16, isOutput=False)
    wq8 = nc.declare_dram_parameter("wq8", [H, P, KO, HD], fp8, isOutput=False)
    wk8 = nc.declare_dram_parameter("wk8", [H, P, KO, HD], fp8, isOutput=False)
    wv = nc.declare_dram_parameter("wv", [P, KO, DPC], bf16, isOutput=False)
    wv8 = nc.declare_dram_parameter("wv8", [P, KO, DPC], fp8, isOutput=False)
    wo = nc.declare_dram_parameter("wo", [P, H, HID], bf16, isOutput=False)
    cosf = nc.declare_dram_parameter("cosf", [P, S], bf16, isOutput=False)
    sinf = nc.declare_dram_parameter("sinf", [P, S], bf16, isOutput=False)
    bmask = nc.declare_dram_parameter("bmask", [P, H, 512], fp8, isOutput=False)
    # bf16 partials: host sums the 4 head-group partials in f32
    out_p = nc.declare_dram_parameter("out_p", [S, HID], bf16, isOutput=True)
    if DBG:
        dbg_qt = nc.declare_dram_parameter("dbg_qt", [P, S], bf16, isOutput=True)
        dbg_kt = nc.declare_dram_parameter("dbg_kt", [P, S], bf16, isOutput=True)
        dbg_vsb = nc.declare_dram_parameter("dbg_vsb", [P, ST, H, 128], fp8,
                                            isOutput=True)
        dbg_pt = nc.declare_dram_parameter("dbg_pt", [P, 2, 512], fp8,
                                           isOutput=True)
        dbg_sm = nc.declare_dram_parameter("dbg_sm", [1, 512], f32, isOutput=True)
        dbg_aot = nc.declare_dram_parameter("dbg_aot", [SC, P, H, 512], bf16,
                                            isOutput=True)

    out3 = out_p.rearrange("(st p) n -> p st n", p=P)

    with tile.TileContext(nc) as tc:
        with ExitStack() as top:
            vpool = top.enter_context(tc.tile_pool(name="vpool", bufs=1))
            qkres = top.enter_context(tc.tile_pool(name="qkres", bufs=1))
            const = top.enter_context(tc.tile_pool(name="const", bufs=1))

            # fp8 V for all 16 tiles (fp8 P@V, chunks 1-3) + fp16 copy of
            # tiles 0-3 for chunk 0's bf16 path
            vsb = vpool.tile([P, ST, H, 128], fp8)
            vsb0 = vpool.tile([P, 2, H, 128], fp16)
            qt_h = [qkres.tile([P, S], bf16, tag=f"qt{h}", name=f"qt{h}")
                    for h in range(H)]
            kt_h = [qkres.tile([P, S], bf16, tag=f"kt{h}", name=f"kt{h}")
                    for h in range(H)]

            zb = const.tile([P, 1], f32)
            nc.vector.memset(zb[:], 0.0)
            nb = const.tile([P, 1], f32)
            nc.vector.memset(nb[:], PB)
            # warm the scalar-engine exp table so the first attention tile
            # doesn't eat the ACT_TABLE_LOAD latency
            warm = const.tile([P, 1], fp16)
            nc.scalar.activation(warm[:], zb[:], AF.Exp, bias=zb[:], scale=1.0)
            bmt = const.tile([P, H, 512], fp8)

            # ---------------- Phase P: projections ----------------
            # fp8 work leads (first matmul gated on ~0.5MB of fp8 stream vs
            # 2.6MB of bf16 under the DMA bandwidth ramp); the bf16 streams
            # land while the fp8 projections compute.
            with ExitStack() as ctx:
                xpool = ctx.enter_context(tc.tile_pool(name="xp", bufs=1))
                wvpool = ctx.enter_context(tc.tile_pool(name="wvp", bufs=1))
                pp = ctx.enter_context(tc.tile_pool(name="pp", bufs=4, space="PSUM"))

                wvq = [wvpool.tile([P, KO // 4, DPC], bf16, tag=f"wv{j}",
                                   name=f"wv{j}") for j in range(4)]
                # fp8 streams: x chunk 1 in quarter tiles (fine-grained DMA
                # watermarks under the ramp), chunks 2-3 whole; wv8 quarters
                # (pair-aligned)
                x8a = xpool.tile([P, KO, 256], fp8, tag="x8a", name="x8a")
                xq1 = [xpool.tile([P, KO // 4, 512], fp8, tag=f"xq1{j}",
                                  name=f"xq1{j}") for j in range(4)]
                xs8 = [None, xpool.tile([P, KO, 512], fp8, tag="x82", name="x82"),
                       xpool.tile([P, KO, 512], fp8, tag="x83", name="x83")]
                wv8q = [wvpool.tile([P, KO // 4, DPC], fp8, tag=f"w8{j}",
                                    name=f"w8{j}") for j in range(4)]
                cspool = ctx.enter_context(tc.tile_pool(name="cs", bufs=1))
                rtmp = ctx.enter_context(tc.tile_pool(name="rt", bufs=3))
                wpool = ctx.enter_context(tc.tile_pool(name="wqk", bufs=4))
                w8pool = ctx.enter_context(tc.tile_pool(name="wqk8", bufs=4))
                cosT = cspool.tile([P, S], bf16)
                sinT = cspool.tile([P, S], bf16)
                xc0 = xpool.tile([P, KO, 256], bf16, tag="xc0", name="xc0")

                # fp8 moving/stationary slice helpers: kp indexes ko-pairs
                def xq1_pair(kp, so=None):
                    t = xq1[kp // 2][:, 2 * (kp % 2):2 * (kp % 2) + 2]
                    return t if so is None else t[:, :, so:so + P]

                def x8_pair(sc, kp, so=None):
                    if sc == 0:
                        t = x8a[:, 2 * kp:2 * kp + 2]
                    elif sc == 1:
                        return xq1_pair(kp, so)
                    else:
                        t = xs8[sc - 1][:, 2 * kp:2 * kp + 2]
                    return t if so is None else t[:, :, so:so + P]

                # critical fp8 set over all three queues
                nc.sync.dma_start(wv8q[0][:], wv8[:, 0:4])
                nc.scalar.dma_start(xq1[0][:], xt8[:, 0:4, 256:768])
                nc.gpsimd.dma_start(wv8q[1][:], wv8[:, 4:8])
                nc.sync.dma_start(xq1[1][:], xt8[:, 4:8, 256:768])
                nc.scalar.dma_start(wv8q[2][:], wv8[:, 8:12])
                nc.gpsimd.dma_start(xq1[2][:], xt8[:, 8:12, 256:768])
                nc.sync.dma_start(wv8q[3][:], wv8[:, 12:16])
                nc.scalar.dma_start(xq1[3][:], xt8[:, 12:16, 256:768])
                nc.gpsimd.dma_start(x8a[:, 0:8], xt8[:, 0:8, 0:256])
                nc.sync.dma_start(x8a[:, 8:16], xt8[:, 8:16, 0:256])

                # V tiles: fp8 DoubleRow (x stationary pair, wv moving)
                def v_block8(sc, sts=None):
                    for st in (sts if sts is not None else range(sc * 4, sc * 4 + 4)):
                        so = (st % 4) * P if sc != 0 else (st - 2) * P
                        ps = pp.tile([P, 512], f32, tag="vproj", name="ps")
                        for kp in range(KO // 2):
                            nc.tensor.matmul(
                                ps[:],
                                x8_pair(sc, kp, so),
                                wv8q[kp // 2][:, 2 * (kp % 2):2 * (kp % 2) + 2],
                                start=(kp == 0),
                                stop=(kp == KO // 2 - 1),
                                perf_mode=DR,
                            )
                        nc.vector.tensor_scalar_mul(
                            vsb[:, st].rearrange("p h d -> p (h d)"),
                            ps[:], 1.0 / WS,
                        )

                v_block8(1)
                v_block8(0, sts=(2, 3))
                nc.sync.dma_start(xs8[1][:, 0:8], xt8[:, 0:8, 768:1280])
                nc.scalar.dma_start(xs8[1][:, 8:16], xt8[:, 8:16, 768:1280])
                nc.gpsimd.dma_start(cosT[:], cosf[:])
                v_block8(2)
                nc.sync.dma_start(xs8[2][:, 0:8], xt8[:, 0:8, 1280:1792])
                nc.scalar.dma_start(xs8[2][:, 8:16], xt8[:, 8:16, 1280:1792])
                nc.gpsimd.dma_start(sinT[:], sinf[:])
                # bf16 streams for the early-query exact path
                nc.sync.dma_start(wvq[0][:], wv[:, 0:4])
                nc.scalar.dma_start(wvq[1][:], wv[:, 4:8])
                nc.gpsimd.dma_start(xc0[:, 0:8], xt[:, 0:8, :])

                v_block8(3)
                nc.sync.dma_start(wvq[2][:], wv[:, 8:12])
                nc.scalar.dma_start(xc0[:, 8:16], xt[:, 8:16, :])
                nc.gpsimd.dma_start(wvq[3][:], wv[:, 12:16])
                nc.gpsimd.dma_start(bmt[:], bmask[:])

                # V tiles 0-1: bf16 from x cols 0:256
                for st in range(2):
                    so = st * P
                    ps = pp.tile([P, 512], f32, tag="vproj", name="ps")
                    for ko in range(KO):
                        nc.tensor.matmul(
                            ps[:],
                            xc0[:, ko, so:so + P],
                            wvq[ko // 4][:, ko % 4],
                            start=(ko == 0),
                            stop=(ko == KO - 1),
                        )
                    # chunk-0 V: fp16 copy (true scale) + fp8 copy, both DVE
                    nc.vector.tensor_scalar_mul(
                        vsb0[:, st].rearrange("p h d -> p (h d)"),
                        ps[:], 1.0 / WS,
                    )
                    nc.vector.tensor_scalar_mul(
                        vsb[:, st].rearrange("p h d -> p (h d)"),
                        ps[:], 1.0 / WS,
                    )

                def rope_evict(ps, dst, ssl, W=512):
                    # RoPE eviction. The rotate's partition swap runs on the
                    # scalar engine (idle during phase P) as two shifted
                    # PSUM->fp16 copies, so the DVE does one full-height fp16
                    # mul instead of two half-height f32-source muls; DVE
                    # per-chunk time drops below the PE's, which was the QK
                    # phase pacer.
                    sw = rtmp.tile([P, 512], fp16, tag="sw")
                    t0 = rtmp.tile([P, 512], fp16, tag="t0")
                    t1 = rtmp.tile([P, 512], fp16, tag="t1")
                    nc.scalar.activation(sw[0:64, 0:W], ps[64:128, 0:W],
                                         AF.Copy)
                    nc.scalar.activation(sw[64:128, 0:W], ps[0:64, 0:W],
                                         AF.Copy)
                    nc.vector.tensor_mul(t0[:, 0:W], sw[:, 0:W], sinT[:, ssl])
                    nc.vector.tensor_mul(t1[:, 0:W], ps[:, 0:W], cosT[:, ssl])
                    nc.vector.tensor_add(dst[:, ssl], t1[:, 0:W], t0[:, 0:W])

                # Q and K per head, head 0 first: attention (c,h) gates on
                # the per-head QT/KT tile's LAST write, so finishing whole
                # heads early lets the attention stream start while later
                # heads' RoPE evictions drain through the DVE queue.
                for h in range(H):
                    for w4, w84, dst_h in ((wq, wq8, qt_h), (wk, wk8, kt_h)):
                        w8t = w8pool.tile([P, KO, HD], fp8, tag="w8")
                        nc.gpsimd.dma_start(w8t[:], w84[h])
                        wt = wpool.tile([P, KO, HD], bf16, tag="w")
                        nc.scalar.dma_start(wt[:], w4[h])
                        # chunks 1-3 + cols 256:512 of chunk 0: fp8 DoubleRow
                        for sc in range(1, SC):
                            ssl = slice(sc * 512, (sc + 1) * 512)
                            ps = pp.tile([P, 512], f32, tag="proj")
                            for kp in range(KO // 2):
                                nc.tensor.matmul(
                                    ps[:],
                                    w8t[:, 2 * kp:2 * kp + 2],
                                    x8_pair(sc, kp),
                                    start=(kp == 0),
                                    stop=(kp == KO // 2 - 1),
                                    perf_mode=DR,
                                )
                            rope_evict(ps, dst_h[h], ssl)
                        ps = pp.tile([P, 512], f32, tag="proj")
                        for kp in range(KO // 2):
                            nc.tensor.matmul(
                                ps[:, 0:256],
                                w8t[:, 2 * kp:2 * kp + 2],
                                x8_pair(0, kp),
                                start=(kp == 0),
                                stop=(kp == KO // 2 - 1),
                                perf_mode=DR,
                            )
                        rope_evict(ps, dst_h[h], slice(256, 512), W=256)
                        # cols 0:256: bf16 exact
                        ps = pp.tile([P, 512], f32, tag="proj")
                        for ko in range(KO):
                            nc.tensor.matmul(
                                ps[:, 0:256],
                                wt[:, ko],
                                xc0[:, ko],
                                start=(ko == 0),
                                stop=(ko == KO - 1),
                            )
                        rope_evict(ps, dst_h[h], slice(0, 256), W=256)

            if DBG:
                nc.sync.dma_start(dbg_qt[:], qt_h[0][:])
                nc.sync.dma_start(dbg_kt[:], kt_h[0][:])
                nc.sync.dma_start(dbg_vsb[:], vsb[:])

            # ------------- Phase A: attention + interleaved o_proj -------------
            with ExitStack() as ctx:
                ppool = ctx.enter_context(tc.tile_pool(name="ppool", bufs=6))
                smpool = ctx.enter_context(tc.tile_pool(name="smp", bufs=2))
                stage = ctx.enter_context(tc.tile_pool(name="stage", bufs=4))
                aopool = ctx.enter_context(tc.tile_pool(name="ao", bufs=1))
                wopool = ctx.enter_context(tc.tile_pool(name="wop", bufs=1))
                ost = ctx.enter_context(tc.tile_pool(name="ost", bufs=4))
                spsum = ctx.enter_context(tc.tile_pool(name="sps", bufs=2, space="PSUM"))
                opsum = ctx.enter_context(tc.tile_pool(name="ops", bufs=2, space="PSUM"))
                smq = ctx.enter_context(tc.tile_pool(name="smq", bufs=2, space="PSUM"))
                opo = ctx.enter_context(tc.tile_pool(name="opo", bufs=2, space="PSUM"))

                # full-width ones: the softmax-sum matmuls write the sum
                # broadcast across all 128 psum partitions, so the normalize
                # chain needs no gpsimd partition_broadcast (slow DSP, and
                # mixing op kinds forces ~7us library swaps)
                ones_col = const.tile([P, P], fp16)
                nc.vector.memset(ones_col[:], 1.0)
                ones8 = const.tile([P, 2, P], fp8)
                nc.vector.memset(ones8[:], 1.0)
                # wot's dma is issued after chunk 0's first head (it would
                # gate c0's first matmuls via the queue watermark otherwise)
                wot = wopool.tile([P, H, HID], bf16)

                aot_c = [
                    aopool.tile([P, H, 512], bf16, tag=f"aot{c}", name=f"aot{c}")
                    for c in range(SC)
                ]

                def emit_oproj(cc):
                    for st4 in range(4):
                        st = cc * 4 + st4
                        for nch in range(4):
                            g = st4 * 4 + nch
                            pso = opo.tile([P, 512], f32, tag="po", name="pso")
                            for dc in range(H):
                                nc.tensor.matmul(
                                    pso[:],
                                    aot_c[cc][:, dc, st4 * P:(st4 + 1) * P],
                                    wot[:, dc, nch * 512:(nch + 1) * 512],
                                    start=(dc == 0),
                                    stop=(dc == H - 1),
                                )
                            # PSUM->SBUF eviction split between scalar ACT
                            # and DVE (gpsimd cannot read PSUM); bf16 out
                            # halves the write stream, spread over 3 queues
                            ob = ost.tile([P, 512], bf16, tag="ob", name="ob")
                            if g % 2 == 0:
                                nc.scalar.activation(ob[:], pso[:], AF.Copy)
                            else:
                                nc.vector.tensor_copy(ob[:], pso[:])
                            eng = (nc.sync, nc.gpsimd, nc.scalar)[g % 3]
                            eng.dma_start(
                                out3[:, st, nch * 512:(nch + 1) * 512], ob[:]
                            )

                # Per-head normalize chain (sm matmul -> rcp -> gpsimd
                # broadcast -> DVE mul), pumped ONE STAGE PER TILE of the
                # following head(s). Emitting the whole chain at once parks
                # ops at the head of the in-order DVE/tensor queues waiting
                # on cross-engine inputs and convoys the tile stream; staged,
                # every op is data-ready when its queue reaches it.
                pending = []

                def norm_pump():
                    if not pending:
                        return
                    e = pending[0]
                    s = e["s"]
                    e["s"] += 1
                    w = e["w"]
                    if s == 0:
                        # fp8 chunks accumulate the softmax sum on the PE
                        # (ones8 DoubleRow per pair) so e["smps"] is already
                        # set; bf16 parts reduce the DVE-accumulated smacc
                        if e["smps"] is None:
                            e["smps"] = smq.tile([P, 512], f32, tag="smp",
                                                 name="smp")
                            nc.tensor.matmul(
                                e["smps"][:, 0:w], ones_col[:],
                                e["sm"][:, 0:w],
                                start=True, stop=True,
                            )
                    elif s == 1:
                        e["rcp"] = stage.tile([P, 512], f32, tag="rcp",
                                              name="rcp")
                        nc.vector.reciprocal_approx_fast(
                            e["rcp"][:, 0:w], e["smps"][:, 0:w])
                    else:
                        qo = e["qo"]
                        nc.vector.tensor_mul(
                            aot_c[e["c"]][:, e["h"], qo:qo + w],
                            e["ob"][:, 0:w], e["rcp"][:, 0:w])
                        pending.pop(0)

                for c in range(SC):
                    qsl = lambda off: slice(c * 512 + off, (c + 1) * 512)
                    nt = 4 * (c + 1)
                    for h in range(H):
                        # finish the chain that owns the recycled ring slot
                        # before reallocating it (only bites in c0's short
                        # 4-tile heads)
                        while len(pending) >= 2:
                            norm_pump()
                        # attn_outT accumulator [d, sq]
                        ob_ps = opsum.tile([P, 512], f32, tag="obp", name="obp")
                        if c == 0:
                            # chunk 0 part A (queries 0:255): bf16/fp16 exact
                            # path (early-query accuracy: fp8 noise doesn't
                            # average over few keys)
                            smacc = smpool.tile([P, 512], fp16, tag="sma",
                                                name="sma")
                            for ti, t in enumerate(range(2)):
                                norm_pump()
                                off = P * t
                                ps = spsum.tile([P, 512], f32, tag="s")
                                nc.tensor.matmul(
                                    ps[:, off:256],
                                    kt_h[h][:, t * P:(t + 1) * P],
                                    qt_h[h][:, off:256],
                                    start=True,
                                    stop=True,
                                )
                                pt = ppool.tile([P, 512], fp16, tag="pt")
                                nc.scalar.activation(
                                    pt[:, off:256], ps[:, off:256], AF.Exp,
                                    bias=zb[:], scale=SCALE,
                                )
                                nc.vector.tensor_mul(
                                    pt[:, off:256], pt[:, off:256],
                                    bmt[:, t, off:256]
                                )
                                nc.tensor.matmul(
                                    ob_ps[:, off:256],
                                    vsb0[:, t, h],
                                    pt[:, off:256],
                                    start=(ti == 0),
                                    stop=(ti == 1),
                                )
                                if ti == 0:
                                    nc.vector.tensor_copy(
                                        smacc[:, 0:256], pt[:, 0:256])
                                else:
                                    nc.vector.tensor_add(
                                        smacc[:, off:256], smacc[:, off:256],
                                        pt[:, off:256],
                                    )
                            pending.append(
                                {"s": 0, "c": 0, "h": h, "ob": ob_ps,
                                 "sm": smacc, "smps": None, "w": 256,
                                 "qo": 0}
                            )
                            # part B (queries 256:511): fp8 pairs over key
                            # tiles (2,3) diagonal then (0,1)
                            while len(pending) >= 2:
                                norm_pump()
                            ob_ps = opsum.tile([P, 512], f32, tag="obp",
                                               name="obp")
                            smq_ps = smq.tile([P, 512], f32, tag="smp",
                                              name="smq_ps")
                            for pi, (ta, tb) in enumerate(((2, 3), (0, 1))):
                                pt8 = ppool.tile([P, 2, 512], fp8, tag="pt")
                                offs = []
                                for i, t in enumerate((ta, tb)):
                                    norm_pump()
                                    r = t - 2
                                    off = P * max(r, 0)
                                    offs.append(off)
                                    ps = spsum.tile([P, 512], f32, tag="s")
                                    nc.tensor.matmul(
                                        ps[:, off:256],
                                        kt_h[h][:, t * P:(t + 1) * P],
                                        qt_h[h][:, 256 + off:512],
                                        start=True,
                                        stop=True,
                                    )
                                    nc.scalar.activation(
                                        pt8[:, i, off:256], ps[:, off:256],
                                        AF.Exp, bias=nb[:], scale=SCALE,
                                    )
                                    if r >= 0:
                                        if off > 0:
                                            nc.vector.memset(
                                                pt8[:, i, 0:off], 0.0)
                                        nc.vector.tensor_mul(
                                            pt8[:, i, off:256],
                                            pt8[:, i, off:256],
                                            bmt[:, r, off:256],
                                        )
                                nc.tensor.matmul(
                                    ob_ps[:, 0:256],
                                    vsb[:, ta:ta + 2, h],
                                    pt8[:, :, 0:256],
                                    start=(pi == 0),
                                    stop=(pi == 1),
                                    perf_mode=DR,
                                )
                                nc.tensor.matmul(
                                    smq_ps[:, 0:256],
                                    ones8[:],
                                    pt8[:, :, 0:256],
                                    start=(pi == 0),
                                    stop=(pi == 1),
                                    perf_mode=DR,
                                )
                            pending.append(
                                {"s": 0, "c": 0, "h": h, "ob": ob_ps,
                                 "sm": None, "smps": smq_ps, "w": 256,
                                 "qo": 256}
                            )
                        else:
                            # chunks 1-3: fp8 P@V in DoubleRow pairs of
                            # adjacent key tiles; probabilities exp(s-2) in
                            # e4m3, causal mask on DVE, softmax sums via
                            # ones8 DoubleRow into a [16,512] psum (row 0).
                            # diagonal pairs first: their exp+mask latency
                            # hides behind the dense unmasked tail of this
                            # head and the previous head's stream
                            smq_ps = smq.tile([P, 512], f32, tag="smp",
                                              name="smq_ps")
                            t_order = list(range(4 * c, nt)) + list(range(0, 4 * c))
                            npair = nt // 2
                            for pi in range(npair):
                                ta, tb = t_order[2 * pi], t_order[2 * pi + 1]
                                pt8 = ppool.tile([P, 2, 512], fp8, tag="pt")
                                offs = []
                                for i, t in enumerate((ta, tb)):
                                    norm_pump()
                                    if c == SC - 1 and h >= 2:
                                        norm_pump()
                                    r = t - 4 * c
                                    off = P * max(r, 0)
                                    offs.append(off)
                                    ps = spsum.tile([P, 512], f32, tag="s")
                                    nc.tensor.matmul(
                                        ps[:, off:512],
                                        kt_h[h][:, t * P:(t + 1) * P],
                                        qt_h[h][:, qsl(off)],
                                        start=True,
                                        stop=True,
                                    )
                                    nc.scalar.activation(
                                        pt8[:, i, off:512], ps[:, off:512],
                                        AF.Exp, bias=nb[:], scale=SCALE,
                                    )
                                    if r >= 0:
                                        # slot 1 of a diagonal pair: the PV
                                        # matmul reads from the pair's base
                                        # offset, so zero the stale region
                                        # below this slot's diagonal (mask
                                        # multiply can't: stale fp8 bytes can
                                        # be NaN encodings and NaN*0=NaN)
                                        offp = P * 2 * pi
                                        if off > offp:
                                            nc.vector.memset(
                                                pt8[:, i, offp:off], 0.0)
                                        nc.vector.tensor_mul(
                                            pt8[:, i, off:512],
                                            pt8[:, i, off:512],
                                            bmt[:, r, off:512],
                                        )
                                off0 = min(offs)
                                nc.tensor.matmul(
                                    ob_ps[:, off0:512],
                                    vsb[:, ta:ta + 2, h],
                                    pt8[:, :, off0:512],
                                    start=(pi == 0),
                                    stop=(pi == npair - 1),
                                    perf_mode=DR,
                                )
                                nc.tensor.matmul(
                                    smq_ps[:, off0:512],
                                    ones8[:],
                                    pt8[:, :, off0:512],
                                    start=(pi == 0),
                                    stop=(pi == npair - 1),
                                    perf_mode=DR,
                                )
                                if DBG and c == 1 and h == 0 and pi == 0:
                                    nc.sync.dma_start(dbg_pt[:], pt8[:])
                            if DBG and c == 1 and h == 0:
                                sdump = stage.tile([1, 512], f32, tag="sdmp",
                                                   name="sdump")
                                nc.vector.tensor_copy(sdump[:], smq_ps[0:1])
                                nc.sync.dma_start(dbg_sm[:], sdump[:])
                        if c > 0:
                            pending.append(
                                {"s": 0, "c": c, "h": h, "ob": ob_ps,
                                 "sm": None, "smps": smq_ps, "w": 512,
                                 "qo": 0}
                            )
                        if c == 0 and h == 0:
                            # issue late so it doesn't gate c0's matmuls
                            nc.gpsimd.dma_start(wot[:], wo[:])

                    # o_proj deferred by one chunk: its aot inputs are then
                    # guaranteed ready, so the PE stream never stalls on the
                    # normalize tail
                    if c > 0:
                        emit_oproj(c - 1)
                while pending:
                    norm_pump()
                emit_oproj(SC - 1)
                if DBG:
                    for c in range(SC):
                        nc.sync.dma_start(dbg_aot[c], aot_c[c][:])

    nc.compile()
    return nc


def _host_prep(hidden_states, position_ids, Wq, Wk, Wv, Wo):
    """Build the 8 per-core input maps (bf16/fp8 weights/activations)."""
    inv_freq = 1.0 / (10000.0 ** (np.arange(0, HD, 2, dtype=np.float32) / HD))
    t = np.arange(S, dtype=np.float32)
    freqs = np.outer(t, inv_freq).astype(np.float32)  # [S, 64]

    bm = np.empty((P, H, 512), dtype=np.float32)
    i = np.arange(P)[:, None, None]
    r = np.arange(H)[None, :, None]
    j = np.arange(512)[None, None, :]
    bm[:] = np.where(i + P * r <= j, 1.0, 0.0)
    bm = bm.astype(ml_dtypes.float8_e4m3)

    in_maps = []
    per_batch = []
    for b in range(B):
        xT = np.ascontiguousarray(hidden_states[b].T)  # [HID, S]
        xt_sw = np.ascontiguousarray(
            xT.reshape(KO, P, S).transpose(1, 0, 2)
        )  # [P, KO, S] f32
        xt_b = np.ascontiguousarray(xt_sw[:, :, 0:256]).astype(
            ml_dtypes.bfloat16)
        xt_8 = np.ascontiguousarray(xt_sw[:, :, 256:2048]).astype(
            ml_dtypes.float8_e4m3)
        fp = freqs[position_ids[b]]  # [S, 64]
        ch = np.cos(fp).T / WS       # [64, S]; 1/32 weight descale folded in
        sh = np.sin(fp).T / WS
        cosf = np.ascontiguousarray(np.concatenate([ch, ch], axis=0)).astype(
            ml_dtypes.bfloat16)   # [128, S]
        sinf = np.ascontiguousarray(np.concatenate([-sh, sh], axis=0)).astype(
            ml_dtypes.bfloat16)  # signed
        per_batch.append((xt_b, xt_8, cosf, sinf))

    for core in range(8):
        b, hg = core // 4, core % 4
        sl = slice(hg * DPC, (hg + 1) * DPC)
        xt_b, xt_8, cosf, sinf = per_batch[b]
        wq_sw = np.ascontiguousarray(
            Wq[sl].T.reshape(KO, P, H, HD).transpose(2, 1, 0, 3)) * WS
        wk_sw = np.ascontiguousarray(
            Wk[sl].T.reshape(KO, P, H, HD).transpose(2, 1, 0, 3)) * WS
        wv_sw = np.ascontiguousarray(
            Wv[sl].T.reshape(KO, P, DPC).transpose(1, 0, 2)) * WS
        wo_sw = np.ascontiguousarray(
            Wo[:, sl].T.reshape(H, HD, HID).transpose(1, 0, 2)
        ).astype(ml_dtypes.bfloat16)  # [P, H, HID]
        in_maps.append({
            "xt": xt_b, "xt8": xt_8,
            "wq": wq_sw.astype(ml_dtypes.bfloat16),
            "wk": wk_sw.astype(ml_dtypes.bfloat16),
            "wq8": wq_sw.astype(ml_dtypes.float8_e4m3),
            "wk8": wk_sw.astype(ml_dtypes.float8_e4m3),
            "wv": wv_sw.astype(ml_dtypes.bfloat16),
            "wv8": wv_sw.astype(ml_dtypes.float8_e4m3),
            "wo": wo_sw,
            "cosf": cosf, "sinf": sinf, "bmask": bm,
        })
    return in_maps


def kernel(hidden_states, attention_mask, position_ids, Wq, Wk, Wv, Wo,
           _trace=False, _trace_kwargs=None):
    global _CACHED_NC
    hidden_states = np.asarray(hidden_states, dtype=np.float32)
    position_ids = np.asarray(position_ids)
    Wq, Wk, Wv, Wo = (np.asarray(w, dtype=np.float32) for w in (Wq, Wk, Wv, Wo))

    if _CACHED_NC is None:
        _CACHED_NC = build_nc()
    nc = _CACHED_NC

    in_maps = _host_prep(hidden_states, position_ids, Wq, Wk, Wv, Wo)
    res = run_bass_kernel_spmd(
        nc, in_maps, list(range(8)), trace=_trace, **(_trace_kwargs or {})
    )

    out = np.empty((B, S, HID), dtype=np.float32)
    for b in range(B):
        acc = res.results[b * 4]["out_p"].astype(np.float32)
        for hg in range(1, 4):
            acc = acc + res.results[b * 4 + hg]["out_p"].astype(np.float32)
        out[b] = acc
    if _trace:
        return out, res
    return out


# revision 37
# speedup vs baseline: 1.1484x; 1.1484x over previous
"""TRN2 Bass kernel for causal multi-head attention with RoPE.

Problem: B=2, S=2048, HID=2048, NH=16, HD=128 (fp32 in/out).
Sharding: 8 cores = 2 (batch) x 4 (head-groups of 4 heads).
Each core computes q/k/v projections for its 4 heads (column-parallel),
RoPE, causal attention, and a row-parallel partial o_proj; the host sums
the 4 partials per batch.

v2 (363us -> ~287us): fp8 (e4m3) DoubleRow matmuls at 2x bf16 PE rate
for the bulk of the work, exploiting the loose 2e-2 rel-err gate
(final rel err ~8.5e-3). Error analysis: softmax here is broad (logit
std ~= 1), so iid fp8 quantization noise on q/k/v/P averages down by
~1/sqrt(N_keys) for late queries; only EARLY queries (few keys) and
the final o_proj see fp8 noise unattenuated. Hence:
  - queries/keys 0..255 run a bf16/fp16 exact path (bf16 q/k/v
    projections, fp16 probabilities, fp16 V copy),
  - everything else uses fp8 DoubleRow for Q/K/V projections and P@V
    in pairs of adjacent key tiles (P stored fp8 with exp bias -2 so
    values fit e4m3's +-240 range - the bias cancels in softmax
    normalization),
  - scores stay bf16: DoubleRow needs the d=128 contraction split to
    K=64, and K=64 DR measures HALF the K=128 rate (417 vs 211 ns per
    512-col matmul) - zero gain, so don't (tried, reverted),
  - o_proj stays bf16 (no averaging after it; fp8 would be ~3.5% err).
Weights are host-prescaled by 32 (W elems ~ N(0, 1/2048) would land in
e4m3's subnormal range); the 1/32 is folded into the RoPE tables (bf16)
and the V-eviction scale.

Engine-placement lessons (measured, not guessed):
  - GpSimd DSP is ~3x slower than DVE per tensor op (1.15us vs ~0.35us
    for a [128,512] fp16 add) and swapping op kinds forces ~7us library
    reloads; it now does ONLY dma_start issue. (Moving rope-adds or
    smacc there cost 50-290us total - reverted.)
  - Softmax sums accumulate on the PE: per-pair ones8 [128,2,128]
    DoubleRow matmuls into a psum whose 128 identical rows make the
    sum pre-broadcast, so the normalize chain is just reciprocal (DVE,
    full height) -> aot multiply. No partition_broadcast at all.
    (DVE smacc adds convoy the mask->PV chain: +80us - reverted.)
  - fp8 stale-byte hazard: pt8 ring slots hold old fp16 bytes that
    alias to e4m3 NaN, and NaN*0=NaN, so diagonal-pair gaps are
    memset to 0 instead of relying on the mask multiply.
  - Phase P runs fp8-first: the first V matmuls gate on ~0.5MB of fp8
    stream instead of 2.6MB of bf16 under the ~20us DMA bandwidth
    ramp; bf16 streams (xc0, wv, wq/wk) land during fp8 compute.
  - Q and K are projected per head, head 0 first, so attention starts
    while later heads' RoPE evictions drain the DVE queue.
  - Do NOT emit chunk 2's o_proj inside chunk 3's head loop: +55us
    (tried, reverted; cause unclear - keep emission after the loop).

Carried over from v1: SBUF-resident per-head QT/KT, 4-deep weight tile
rings, staged per-head normalize chain (one stage pumped per tile of
the following head), o_proj deferred one chunk and its bf16 partials
spread over 3 DMA queues.
"""
import os
import sys

if "/opt/trn_rl_repo" not in sys.path:
    sys.path.insert(0, "/opt/trn_rl_repo")

import numpy as np
import ml_dtypes

import concourse.bass as bass
import concourse.mybir as mybir
import concourse.tile as tile
from concourse import bacc
from concourse.bass_utils import run_bass_kernel_spmd
from contextlib import ExitStack

P = 128
B, S, HID, NH = 2, 2048, 2048, 16
HD = HID // NH              # 128
H = 4                       # heads per core
DPC = H * HD                # 512 dims per core
KO = HID // P               # 16 contraction chunks
SC = S // 512               # 4 seq chunks of 512
ST = S // P                 # 16 seq tiles of 128
SCALE = 1.0 / float(np.sqrt(HD))
WS = 32.0                   # host weight prescale (fp8 range)
PB = -2.0                   # exp bias for fp8 P (cancels in softmax)

f32 = mybir.dt.float32
bf16 = mybir.dt.bfloat16
fp16 = mybir.dt.float16
fp8 = mybir.dt.float8e4
DR = mybir.MatmulPerfMode.DoubleRow

_CACHED_NC = None


def build_nc():
    DBG = os.environ.get("KDBG", "0") == "1"
    AF = mybir.ActivationFunctionType
    nc = bacc.Bacc(None, target_bir_lowering=False)

    # chunk 0 of x in bf16 (early-query exact path); chunks 1-3 fp8
    xt = nc.declare_dram_parameter("xt", [P, KO, 256], bf16, isOutput=False)
    xt8 = nc.declare_dram_parameter("xt8", [P, KO, 1792], fp8, isOutput=False)
    wq = nc.declare_dram_parameter("wq", [H, P, KO, HD], bf16, isOutput=False)
    wk = nc.declare_dram_parameter("wk", [H, P, KO, HD], bf16, isOutput=False)
    wq8 = nc.declare_dram_parameter("wq8", [H, P, KO, HD], fp8, isOutput=False)
    wk8 = nc.declare_dram_parameter("wk8", [H, P, KO, HD], fp8, isOutput=False)
    wv = nc.declare_dram_parameter("wv", [P, KO, DPC], bf16, isOutput=False)
    wv8 = nc.declare_dram_parameter("wv8", [P, KO, DPC], fp8, isOutput=False)
    wo = nc.declare_dram_parameter("wo", [P, H, HID], bf16, isOutput=False)
    cosf = nc.declare_dram_parameter("cosf", [P, S], bf16, isOutput=False)
    sinf = nc.declare_dram_parameter("sinf", [P, S], bf16, isOutput=False)
    bmask = nc.declare_dram_parameter("bmask", [P, H, 512], fp8, isOutput=False)
    # bf16 partials: host sums the 4 head-group partials in f32
    out_p = nc.declare_dram_parameter("out_p", [S, HID], bf16, isOutput=True)
    if DBG:
        dbg_qt = nc.declare_dram_parameter("dbg_qt", [P, S], bf16, isOutput=True)
        dbg_kt = nc.declare_dram_parameter("dbg_kt", [P, S], bf16, isOutput=True)
        dbg_vsb = nc.declare_dram_parameter("dbg_vsb", [P, ST, H, 128], fp8,
                                            isOutput=True)
        dbg_pt = nc.declare_dram_parameter("dbg_pt", [P, 2, 512], fp8,
                                           isOutput=True)
        dbg_sm = nc.declare_dram_parameter("dbg_sm", [1, 512], f32, isOutput=True)
        dbg_aot = nc.declare_dram_parameter("dbg_aot", [SC, P, H, 512], bf16,
                                            isOutput=True)

    out3 = out_p.rearrange("(st p) n -> p st n", p=P)

    with tile.TileContext(nc) as tc:
        with ExitStack() as top:
            vpool = top.enter_context(tc.tile_pool(name="vpool", bufs=1))
            qkres = top.enter_context(tc.tile_pool(name="qkres", bufs=1))
            const = top.enter_context(tc.tile_pool(name="const", bufs=1))

            # fp8 V for all 16 tiles (fp8 P@V, chunks 1-3) + fp16 copy of
            # tiles 0-3 for chunk 0's bf16 path
            vsb = vpool.tile([P, ST, H, 128], fp8)
            vsb0 = vpool.tile([P, 2, H, 128], fp16)
            qt_h = [qkres.tile([P, S], bf16, tag=f"qt{h}", name=f"qt{h}")
                    for h in range(H)]
            kt_h = [qkres.tile([P, S], bf16, tag=f"kt{h}", name=f"kt{h}")
                    for h in range(H)]

            zb = const.tile([P, 1], f32)
            nc.vector.memset(zb[:], 0.0)
            nb = const.tile([P, 1], f32)
            nc.vector.memset(nb[:], PB)
            # warm the scalar-engine exp table so the first attention tile
            # doesn't eat the ACT_TABLE_LOAD latency
            warm = const.tile([P, 1], fp16)
            nc.scalar.activation(warm[:], zb[:], AF.Exp, bias=zb[:], scale=1.0)
            bmt = const.tile([P, H, 512], fp8)

            # ---------------- Phase P: projections ----------------
            # fp8 work leads (first matmul gated on ~0.5MB of fp8 stream vs
            # 2.6MB of bf16 under the DMA bandwidth ramp); the bf16 streams
            # land while the fp8 projections compute.
            with ExitStack() as ctx:
                xpool = ctx.enter_context(tc.tile_pool(name="xp", bufs=1))
                wvpool = ctx.enter_context(tc.tile_pool(name="wvp", bufs=1))
                pp = ctx.enter_context(tc.tile_pool(name="pp", bufs=4, space="PSUM"))

                wvq = [wvpool.tile([P, KO // 4, DPC], bf16, tag=f"wv{j}",
                                   name=f"wv{j}") for j in range(4)]
                # fp8 streams: x chunk 1 in quarter tiles (fine-grained DMA
                # watermarks under the ramp), chunks 2-3 whole; wv8 quarters
                # (pair-aligned)
                x8a = xpool.tile([P, KO, 256], fp8, tag="x8a", name="x8a")
                xq1 = [xpool.tile([P, KO // 4, 512], fp8, tag=f"xq1{j}",
                                  name=f"xq1{j}") for j in range(4)]
                xs8 = [None, xpool.tile([P, KO, 512], fp8, tag="x82", name="x82"),
                       xpool.tile([P, KO, 512], fp8, tag="x83", name="x83")]
                wv8q = [wvpool.tile([P, KO // 4, DPC], fp8, tag=f"w8{j}",
                                    name=f"w8{j}") for j in range(4)]
                cspool = ctx.enter_context(tc.tile_pool(name="cs", bufs=1))
                rtmp = ctx.enter_context(tc.tile_pool(name="rt", bufs=3))
                wpool = ctx.enter_context(tc.tile_pool(name="wqk", bufs=4))
                w8pool = ctx.enter_context(tc.tile_pool(name="wqk8", bufs=4))
                cosT = cspool.tile([P, S], bf16)
                sinT = cspool.tile([P, S], bf16)
                xc0 = xpool.tile([P, KO, 256], bf16, tag="xc0", name="xc0")

                # fp8 moving/stationary slice helpers: kp indexes ko-pairs
                def xq1_pair(kp, so=None):
                    t = xq1[kp // 2][:, 2 * (kp % 2):2 * (kp % 2) + 2]
                    return t if so is None else t[:, :, so:so + P]

                def x8_pair(sc, kp, so=None):
                    if sc == 0:
                        t = x8a[:, 2 * kp:2 * kp + 2]
                    elif sc == 1:
                        return xq1_pair(kp, so)
                    else:
                        t = xs8[sc - 1][:, 2 * kp:2 * kp + 2]
                    return t if so is None else t[:, :, so:so + P]

                # critical fp8 set over all three queues
                nc.sync.dma_start(wv8q[0][:], wv8[:, 0:4])
                nc.scalar.dma_start(xq1[0][:], xt8[:, 0:4, 256:768])
                nc.gpsimd.dma_start(wv8q[1][:], wv8[:, 4:8])
                nc.sync.dma_start(xq1[1][:], xt8[:, 4:8, 256:768])
                nc.scalar.dma_start(wv8q[2][:], wv8[:, 8:12])
                nc.gpsimd.dma_start(xq1[2][:], xt8[:, 8:12, 256:768])
                nc.sync.dma_start(wv8q[3][:], wv8[:, 12:16])
                nc.scalar.dma_start(xq1[3][:], xt8[:, 12:16, 256:768])
                nc.gpsimd.dma_start(x8a[:, 0:8], xt8[:, 0:8, 0:256])
                nc.sync.dma_start(x8a[:, 8:16], xt8[:, 8:16, 0:256])

                # V tiles: fp8 DoubleRow (x stationary pair, wv moving)
                def v_block8(sc, sts=None):
                    for st in (sts if sts is not None else range(sc * 4, sc * 4 + 4)):
                        so = (st % 4) * P if sc != 0 else (st - 2) * P
                        ps = pp.tile([P, 512], f32, tag="vproj", name="ps")
                        for kp in range(KO // 2):
                            nc.tensor.matmul(
                                ps[:],
                                x8_pair(sc, kp, so),
                                wv8q[kp // 2][:, 2 * (kp % 2):2 * (kp % 2) + 2],
                                start=(kp == 0),
                                stop=(kp == KO // 2 - 1),
                                perf_mode=DR,
                            )
                        nc.vector.tensor_scalar_mul(
                            vsb[:, st].rearrange("p h d -> p (h d)"),
                            ps[:], 1.0 / WS,
                        )

                v_block8(1)
                v_block8(0, sts=(2, 3))
                nc.sync.dma_start(xs8[1][:, 0:8], xt8[:, 0:8, 768:1280])
                nc.scalar.dma_start(xs8[1][:, 8:16], xt8[:, 8:16, 768:1280])
                nc.gpsimd.dma_start(cosT[:], cosf[:])
                v_block8(2)
                nc.sync.dma_start(xs8[2][:, 0:8], xt8[:, 0:8, 1280:1792])
                nc.scalar.dma_start(xs8[2][:, 8:16], xt8[:, 8:16, 1280:1792])
                nc.gpsimd.dma_start(sinT[:], sinf[:])
                # bf16 streams for the early-query exact path
                nc.sync.dma_start(wvq[0][:], wv[:, 0:4])
                nc.scalar.dma_start(wvq[1][:], wv[:, 4:8])
                nc.gpsimd.dma_start(xc0[:, 0:8], xt[:, 0:8, :])

                v_block8(3)
                nc.sync.dma_start(wvq[2][:], wv[:, 8:12])
                nc.scalar.dma_start(xc0[:, 8:16], xt[:, 8:16, :])
                nc.gpsimd.dma_start(wvq[3][:], wv[:, 12:16])
                nc.gpsimd.dma_start(bmt[:], bmask[:])

                # V tiles 0-1: bf16 from x cols 0:256
                for st in range(2):
                    so = st * P
                    ps = pp.tile([P, 512], f32, tag="vproj", name="ps")
                    for ko in range(KO):
                        nc.tensor.matmul(
                            ps[:],
                            xc0[:, ko, so:so + P],
                            wvq[ko // 4][:, ko % 4],
                            start=(ko == 0),
                            stop=(ko == KO - 1),
                        )
                    # chunk-0 V: fp16 copy (true scale) + fp8 copy, both DVE
                    nc.vector.tensor_scalar_mul(
                        vsb0[:, st].rearrange("p h d -> p (h d)"),
                        ps[:], 1.0 / WS,
                    )
                    nc.vector.tensor_scalar_mul(
                        vsb[:, st].rearrange("p h d -> p (h d)"),
                        ps[:], 1.0 / WS,
                    )

                def rope_evict(ps, dst, ssl, W=512):
                    # RoPE eviction. The rotate's partition swap runs on the
                    # scalar engine (idle during phase P) as two shifted
                    # PSUM->fp16 copies, so the DVE does one full-height fp16
                    # mul instead of two half-height f32-source muls; DVE
                    # per-chunk time drops below the PE's, which was the QK
                    # phase pacer.
                    sw = rtmp.tile([P, 512], fp16, tag="sw")
                    t0 = rtmp.tile([P, 512], fp16, tag="t0")
                    t1 = rtmp.tile([P, 512], fp16, tag="t1")
                    nc.scalar.activation(sw[0:64, 0:W], ps[64:128, 0:W],
                                         AF.Copy)
                    nc.scalar.activation(sw[64:128, 0:W], ps[0:64, 0:W],
                                         AF.Copy)
                    nc.vector.tensor_mul(t0[:, 0:W], sw[:, 0:W], sinT[:, ssl])
                    nc.vector.tensor_mul(t1[:, 0:W], ps[:, 0:W], cosT[:, ssl])
                    nc.vector.tensor_add(dst[:, ssl], t1[:, 0:W], t0[:, 0:W])

                # Q and K per head, head 0 first: attention (c,h) gates on
                # the per-head QT/KT tile's LAST write, so finishing whole
                # heads early lets the attention stream start while later
                # heads' RoPE evictions drain through the DVE queue.
                for h in range(H):
                    for w4, w84, dst_h in ((wq, wq8, qt_h), (wk, wk8, kt_h)):
                        w8t = w8pool.tile([P, KO, HD], fp8, tag="w8")
                        nc.gpsimd.dma_start(w8t[:], w84[h])
                        wt = wpool.tile([P, KO, HD], bf16, tag="w")
                        nc.scalar.dma_start(wt[:], w4[h])
                        # chunks 1-3 + cols 256:512 of chunk 0: fp8 DoubleRow
                        for sc in range(1, SC):
                            ssl = slice(sc * 512, (sc + 1) * 512)
                            ps = pp.tile([P, 512], f32, tag="proj")
                            for kp in range(KO // 2):
                                nc.tensor.matmul(
                                    ps[:],
                                    w8t[:, 2 * kp:2 * kp + 2],
                                    x8_pair(sc, kp),
                                    start=(kp == 0),
                                    stop=(kp == KO // 2 - 1),
                                    perf_mode=DR,
                                )
                            rope_evict(ps, dst_h[h], ssl)
                        ps = pp.tile([P, 512], f32, tag="proj")
                        for kp in range(KO // 2):
                            nc.tensor.matmul(
                                ps[:, 0:256],
                                w8t[:, 2 * kp:2 * kp + 2],
                                x8_pair(0, kp),
                                start=(kp == 0),
                                stop=(kp == KO // 2 - 1),
                                perf_mode=DR,
                            )
                        rope_evict(ps, dst_h[h], slice(256, 512), W=256)
                        # cols 0:256: bf16 exact
                        ps = pp.tile([P, 512], f32, tag="proj")
                        for ko in range(KO):
                            nc.tensor.matmul(
                                ps[:, 0:256],
                                wt[:, ko],
                                xc0[:, ko],
                                start=(ko == 0),
                                stop=(ko == KO - 1),
                            )
                        rope_evict(ps, dst_h[h], slice(0, 256), W=256)

            if DBG:
                nc.sync.dma_start(dbg_qt[:], qt_h[0][:])
                nc.sync.dma_start(dbg_kt[:], kt_h[0][:])
                nc.sync.dma_start(dbg_vsb[:], vsb[:])

            # ------------- Phase A: attention + interleaved o_proj -------------
            with ExitStack() as ctx:
                ppool = ctx.enter_context(tc.tile_pool(name="ppool", bufs=6))
                smpool = ctx.enter_context(tc.tile_pool(name="smp", bufs=2))
                stage = ctx.enter_context(tc.tile_pool(name="stage", bufs=4))
                aopool = ctx.enter_context(tc.tile_pool(name="ao", bufs=1))
                wopool = ctx.enter_context(tc.tile_pool(name="wop", bufs=1))
                ost = ctx.enter_context(tc.tile_pool(name="ost", bufs=4))
                spsum = ctx.enter_context(tc.tile_pool(name="sps", bufs=2, space="PSUM"))
                opsum = ctx.enter_context(tc.tile_pool(name="ops", bufs=2, space="PSUM"))
                smq = ctx.enter_context(tc.tile_pool(name="smq", bufs=2, space="PSUM"))
                opo = ctx.enter_context(tc.tile_pool(name="opo", bufs=2, space="PSUM"))

                # full-width ones: the softmax-sum matmuls write the sum
                # broadcast across all 128 psum partitions, so the normalize
                # chain needs no gpsimd partition_broadcast (slow DSP, and
                # mixing op kinds forces ~7us library swaps)
                ones_col = const.tile([P, P], fp16)
                nc.vector.memset(ones_col[:], 1.0)
                ones8 = const.tile([P, 2, P], fp8)
                nc.vector.memset(ones8[:], 1.0)
                # wot's dma is issued after chunk 0's first head (it would
                # gate c0's first matmuls via the queue watermark otherwise)
                wot = wopool.tile([P, H, HID], bf16)

                aot_c = [
                    aopool.tile([P, H, 512], bf16, tag=f"aot{c}", name=f"aot{c}")
                    for c in range(SC)
                ]

                def emit_oproj(cc):
                    for st4 in range(4):
                        st = cc * 4 + st4
                        for nch in range(4):
                            g = st4 * 4 + nch
                            pso = opo.tile([P, 512], f32, tag="po", name="pso")
                            for dc in range(H):
                                nc.tensor.matmul(
                                    pso[:],
                                    aot_c[cc][:, dc, st4 * P:(st4 + 1) * P],
                                    wot[:, dc, nch * 512:(nch + 1) * 512],
                                    start=(dc == 0),
                                    stop=(dc == H - 1),
                                )
                            # PSUM->SBUF eviction split between scalar ACT
                            # and DVE (gpsimd cannot read PSUM); bf16 out
                            # halves the write stream, spread over 3 queues
                            ob = ost.tile([P, 512], bf16, tag="ob", name="ob")
                            if g % 2 == 0:
                                nc.scalar.activation(ob[:], pso[:], AF.Copy)
                            else:
                                nc.vector.tensor_copy(ob[:], pso[:])
                            eng = (nc.sync, nc.gpsimd, nc.scalar)[g % 3]
                            eng.dma_start(
                                out3[:, st, nch * 512:(nch + 1) * 512], ob[:]
                            )

                # Per-head normalize chain (sm matmul -> rcp -> gpsimd
                # broadcast -> DVE mul), pumped ONE STAGE PER TILE of the
                # following head(s). Emitting the whole chain at once parks
                # ops at the head of the in-order DVE/tensor queues waiting
                # on cross-engine inputs and convoys the tile stream; staged,
                # every op is data-ready when its queue reaches it.
                pending = []

                def norm_pump():
                    if not pending:
                        return
                    e = pending[0]
                    s = e["s"]
                    e["s"] += 1
                    w = e["w"]
                    if s == 0:
                        # fp8 chunks accumulate the softmax sum on the PE
                        # (ones8 DoubleRow per pair) so e["smps"] is already
                        # set; bf16 parts reduce the DVE-accumulated smacc
                        if e["smps"] is None:
                            e["smps"] = smq.tile([P, 512], f32, tag="smp",
                                                 name="smp")
                            nc.tensor.matmul(
                                e["smps"][:, 0:w], ones_col[:],
                                e["sm"][:, 0:w],
                                start=True, stop=True,
                            )
                    elif s == 1:
                        e["rcp"] = stage.tile([P, 512], f32, tag="rcp",
                                              name="rcp")
                        nc.vector.reciprocal_approx_fast(
                            e["rcp"][:, 0:w], e["smps"][:, 0:w])
                    else:
                        qo = e["qo"]
                        nc.vector.tensor_mul(
                            aot_c[e["c"]][:, e["h"], qo:qo + w],
                            e["ob"][:, 0:w], e["rcp"][:, 0:w])
                        pending.pop(0)

                corder = [1, 2, 3, 0]
                for ci, c in enumerate(corder):
                    qsl = lambda off: slice(c * 512 + off, (c + 1) * 512)
                    nt = 4 * (c + 1)
                    for h in range(H):
                        # finish the chain that owns the recycled ring slot
                        # before reallocating it (only bites in c0's short
                        # 4-tile heads)
                        while len(pending) >= 2:
                            norm_pump()
                        # attn_outT accumulator [d, sq]
                        ob_ps = opsum.tile([P, 512], f32, tag="obp", name="obp")
                        if c == 0:
                            # chunk 0 part A (queries 0:255): bf16/fp16 exact
                            # path (early-query accuracy: fp8 noise doesn't
                            # average over few keys)
                            smacc = smpool.tile([P, 512], fp16, tag="sma",
                                                name="sma")
                            for ti, t in enumerate(range(2)):
                                norm_pump()
                                off = P * t
                                ps = spsum.tile([P, 512], f32, tag="s")
                                nc.tensor.matmul(
                                    ps[:, off:256],
                                    kt_h[h][:, t * P:(t + 1) * P],
                                    qt_h[h][:, off:256],
                                    start=True,
                                    stop=True,
                                )
                                pt = ppool.tile([P, 512], fp16, tag="pt")
                                nc.scalar.activation(
                                    pt[:, off:256], ps[:, off:256], AF.Exp,
                                    bias=zb[:], scale=SCALE,
                                )
                                nc.vector.tensor_mul(
                                    pt[:, off:256], pt[:, off:256],
                                    bmt[:, t, off:256]
                                )
                                nc.tensor.matmul(
                                    ob_ps[:, off:256],
                                    vsb0[:, t, h],
                                    pt[:, off:256],
                                    start=(ti == 0),
                                    stop=(ti == 1),
                                )
                                if ti == 0:
                                    nc.vector.tensor_copy(
                                        smacc[:, 0:256], pt[:, 0:256])
                                else:
                                    nc.vector.tensor_add(
                                        smacc[:, off:256], smacc[:, off:256],
                                        pt[:, off:256],
                                    )
                            pending.append(
                                {"s": 0, "c": 0, "h": h, "ob": ob_ps,
                                 "sm": smacc, "smps": None, "w": 256,
                                 "qo": 0}
                            )
                            # part B (queries 256:511): fp8 pairs over key
                            # tiles (2,3) diagonal then (0,1)
                            while len(pending) >= 2:
                                norm_pump()
                            ob_ps = opsum.tile([P, 512], f32, tag="obp",
                                               name="obp")
                            smq_ps = smq.tile([P, 512], f32, tag="smp",
                                              name="smq_ps")
                            for pi, (ta, tb) in enumerate(((2, 3), (0, 1))):
                                pt8 = ppool.tile([P, 2, 512], fp8, tag="pt")
                                offs = []
                                for i, t in enumerate((ta, tb)):
                                    norm_pump()
                                    r = t - 2
                                    off = P * max(r, 0)
                                    offs.append(off)
                                    ps = spsum.tile([P, 512], f32, tag="s")
                                    nc.tensor.matmul(
                                        ps[:, off:256],
                                        kt_h[h][:, t * P:(t + 1) * P],
                                        qt_h[h][:, 256 + off:512],
                                        start=True,
                                        stop=True,
                                    )
                                    nc.scalar.activation(
                                        pt8[:, i, off:256], ps[:, off:256],
                                        AF.Exp, bias=nb[:], scale=SCALE,
                                    )
                                    if r >= 0:
                                        if off > 0:
                                            nc.vector.memset(
                                                pt8[:, i, 0:off], 0.0)
                                        nc.vector.tensor_mul(
                                            pt8[:, i, off:256],
                                            pt8[:, i, off:256],
                                            bmt[:, r, off:256],
                                        )
                                nc.tensor.matmul(
                                    ob_ps[:, 0:256],
                                    vsb[:, ta:ta + 2, h],
                                    pt8[:, :, 0:256],
                                    start=(pi == 0),
                                    stop=(pi == 1),
                                    perf_mode=DR,
                                )
                                nc.tensor.matmul(
                                    smq_ps[:, 0:256],
                                    ones8[:],
                                    pt8[:, :, 0:256],
                                    start=(pi == 0),
                                    stop=(pi == 1),
                                    perf_mode=DR,
                                )
                            pending.append(
                                {"s": 0, "c": 0, "h": h, "ob": ob_ps,
                                 "sm": None, "smps": smq_ps, "w": 256,
                                 "qo": 256}
                            )
                        else:
                            # chunks 1-3: fp8 P@V in DoubleRow pairs of
                            # adjacent key tiles; probabilities exp(s-2) in
                            # e4m3, causal mask on DVE, softmax sums via
                            # ones8 DoubleRow into a [16,512] psum (row 0).
                            # diagonal pairs first: their exp+mask latency
                            # hides behind the dense unmasked tail of this
                            # head and the previous head's stream
                            smq_ps = smq.tile([P, 512], f32, tag="smp",
                                              name="smq_ps")
                            t_order = list(range(4 * c, nt)) + list(range(0, 4 * c))
                            npair = nt // 2
                            for pi in range(npair):
                                ta, tb = t_order[2 * pi], t_order[2 * pi + 1]
                                pt8 = ppool.tile([P, 2, 512], fp8, tag="pt")
                                offs = []
                                for i, t in enumerate((ta, tb)):
                                    norm_pump()
                                    if ci == SC - 1 and h >= 2:
                                        norm_pump()
                                    r = t - 4 * c
                                    off = P * max(r, 0)
                                    offs.append(off)
                                    ps = spsum.tile([P, 512], f32, tag="s")
                                    nc.tensor.matmul(
                                        ps[:, off:512],
                                        kt_h[h][:, t * P:(t + 1) * P],
                                        qt_h[h][:, qsl(off)],
                                        start=True,
                                        stop=True,
                                    )
                                    nc.scalar.activation(
                                        pt8[:, i, off:512], ps[:, off:512],
                                        AF.Exp, bias=nb[:], scale=SCALE,
                                    )
                                    if r >= 0:
                                        # slot 1 of a diagonal pair: the PV
                                        # matmul reads from the pair's base
                                        # offset, so zero the stale region
                                        # below this slot's diagonal (mask
                                        # multiply can't: stale fp8 bytes can
                                        # be NaN encodings and NaN*0=NaN)
                                        offp = P * 2 * pi
                                        if off > offp:
                                            nc.vector.memset(
                                                pt8[:, i, offp:off], 0.0)
                                        nc.vector.tensor_mul(
                                            pt8[:, i, off:512],
                                            pt8[:, i, off:512],
                                            bmt[:, r, off:512],
                                        )
                                off0 = min(offs)
                                nc.tensor.matmul(
                                    ob_ps[:, off0:512],
                                    vsb[:, ta:ta + 2, h],
                                    pt8[:, :, off0:512],
                                    start=(pi == 0),
                                    stop=(pi == npair - 1),
                                    perf_mode=DR,
                                )
                                nc.tensor.matmul(
                                    smq_ps[:, off0:512],
                                    ones8[:],
                                    pt8[:, :, off0:512],
                                    start=(pi == 0),
                                    stop=(pi == npair - 1),
                                    perf_mode=DR,
                                )
                                if DBG and c == 1 and h == 0 and pi == 0:
                                    nc.sync.dma_start(dbg_pt[:], pt8[:])
                            if DBG and c == 1 and h == 0:
                                sdump = stage.tile([1, 512], f32, tag="sdmp",
                                                   name="sdump")
                                nc.vector.tensor_copy(sdump[:], smq_ps[0:1])
                                nc.sync.dma_start(dbg_sm[:], sdump[:])
                        if c > 0:
                            pending.append(
                                {"s": 0, "c": c, "h": h, "ob": ob_ps,
                                 "sm": None, "smps": smq_ps, "w": 512,
                                 "qo": 0}
                            )
                        if ci == 0 and h == 0:
                            # issue late so it doesn't gate the first
                            # chunk's matmuls via the queue watermark
                            nc.gpsimd.dma_start(wot[:], wo[:])

                    # o_proj deferred by one chunk: its aot inputs are then
                    # guaranteed ready, so the PE stream never stalls on the
                    # normalize tail
                    if ci > 0:
                        emit_oproj(corder[ci - 1])
                while pending:
                    norm_pump()
                emit_oproj(corder[-1])
                if DBG:
                    for c in range(SC):
                        nc.sync.dma_start(dbg_aot[c], aot_c[c][:])

    nc.compile()
    return nc


def _host_prep(hidden_states, position_ids, Wq, Wk, Wv, Wo):
    """Build the 8 per-core input maps (bf16/fp8 weights/activations)."""
    inv_freq = 1.0 / (10000.0 ** (np.arange(0, HD, 2, dtype=np.float32) / HD))
    t = np.arange(S, dtype=np.float32)
    freqs = np.outer(t, inv_freq).astype(np.float32)  # [S, 64]

    bm = np.empty((P, H, 512), dtype=np.float32)
    i = np.arange(P)[:, None, None]
    r = np.arange(H)[None, :, None]
    j = np.arange(512)[None, None, :]
    bm[:] = np.where(i + P * r <= j, 1.0, 0.0)
    bm = bm.astype(ml_dtypes.float8_e4m3)

    in_maps = []
    per_batch = []
    for b in range(B):
        xT = np.ascontiguousarray(hidden_states[b].T)  # [HID, S]
        xt_sw = np.ascontiguousarray(
            xT.reshape(KO, P, S).transpose(1, 0, 2)
        )  # [P, KO, S] f32
        xt_b = np.ascontiguousarray(xt_sw[:, :, 0:256]).astype(
            ml_dtypes.bfloat16)
        xt_8 = np.ascontiguousarray(xt_sw[:, :, 256:2048]).astype(
            ml_dtypes.float8_e4m3)
        fp = freqs[position_ids[b]]  # [S, 64]
        ch = np.cos(fp).T / WS       # [64, S]; 1/32 weight descale folded in
        sh = np.sin(fp).T / WS
        cosf = np.ascontiguousarray(np.concatenate([ch, ch], axis=0)).astype(
            ml_dtypes.bfloat16)   # [128, S]
        sinf = np.ascontiguousarray(np.concatenate([-sh, sh], axis=0)).astype(
            ml_dtypes.bfloat16)  # signed
        per_batch.append((xt_b, xt_8, cosf, sinf))

    for core in range(8):
        b, hg = core // 4, core % 4
        sl = slice(hg * DPC, (hg + 1) * DPC)
        xt_b, xt_8, cosf, sinf = per_batch[b]
        wq_sw = np.ascontiguousarray(
            Wq[sl].T.reshape(KO, P, H, HD).transpose(2, 1, 0, 3)) * WS
        wk_sw = np.ascontiguousarray(
            Wk[sl].T.reshape(KO, P, H, HD).transpose(2, 1, 0, 3)) * WS
        wv_sw = np.ascontiguousarray(
            Wv[sl].T.reshape(KO, P, DPC).transpose(1, 0, 2)) * WS
        wo_sw = np.ascontiguousarray(
            Wo[:, sl].T.reshape(H, HD, HID).transpose(1, 0, 2)
        ).astype(ml_dtypes.bfloat16)  # [P, H, HID]
        in_maps.append({
            "xt": xt_b, "xt8": xt_8,
            "wq": wq_sw.astype(ml_dtypes.bfloat16),
            "wk": wk_sw.astype(ml_dtypes.bfloat16),
            "wq8": wq_sw.astype(ml_dtypes.float8_e4m3),
            "wk8": wk_sw.astype(ml_dtypes.float8_e4m3),
            "wv": wv_sw.astype(ml_dtypes.bfloat16),
            "wv8": wv_sw.astype(ml_dtypes.float8_e4m3),
            "wo": wo_sw,
            "cosf": cosf, "sinf": sinf, "bmask": bm,
        })
    return in_maps


def kernel(hidden_states, attention_mask, position_ids, Wq, Wk, Wv, Wo,
           _trace=False, _trace_kwargs=None):
    global _CACHED_NC
    hidden_states = np.asarray(hidden_states, dtype=np.float32)
    position_ids = np.asarray(position_ids)
    Wq, Wk, Wv, Wo = (np.asarray(w, dtype=np.float32) for w in (Wq, Wk, Wv, Wo))

    if _CACHED_NC is None:
        _CACHED_NC = build_nc()
    nc = _CACHED_NC

    in_maps = _host_prep(hidden_states, position_ids, Wq, Wk, Wv, Wo)
    res = run_bass_kernel_spmd(
        nc, in_maps, list(range(8)), trace=_trace, **(_trace_kwargs or {})
    )

    out = np.empty((B, S, HID), dtype=np.float32)
    for b in range(B):
        acc = res.results[b * 4]["out_p"].astype(np.float32)
        for hg in range(1, 4):
            acc = acc + res.results[b * 4 + hg]["out_p"].astype(np.float32)
        out[b] = acc
    if _trace:
        return out, res
    return out


# revision 38
# speedup vs baseline: 1.1664x; 1.0157x over previous
"""TRN2 Bass kernel for causal multi-head attention with RoPE.

Problem: B=2, S=2048, HID=2048, NH=16, HD=128 (fp32 in/out).
Sharding: 8 cores = 2 (batch) x 4 (head-groups of 4 heads).
Each core computes q/k/v projections for its 4 heads (column-parallel),
RoPE, causal attention, and a row-parallel partial o_proj; the host sums
the 4 partials per batch.

v2 (363us -> ~287us): fp8 (e4m3) DoubleRow matmuls at 2x bf16 PE rate
for the bulk of the work, exploiting the loose 2e-2 rel-err gate
(final rel err ~8.5e-3). Error analysis: softmax here is broad (logit
std ~= 1), so iid fp8 quantization noise on q/k/v/P averages down by
~1/sqrt(N_keys) for late queries; only EARLY queries (few keys) and
the final o_proj see fp8 noise unattenuated. Hence:
  - queries/keys 0..255 run a bf16/fp16 exact path (bf16 q/k/v
    projections, fp16 probabilities, fp16 V copy),
  - everything else uses fp8 DoubleRow for Q/K/V projections and P@V
    in pairs of adjacent key tiles (P stored fp8 with exp bias -2 so
    values fit e4m3's +-240 range - the bias cancels in softmax
    normalization),
  - scores stay bf16: DoubleRow needs the d=128 contraction split to
    K=64, and K=64 DR measures HALF the K=128 rate (417 vs 211 ns per
    512-col matmul) - zero gain, so don't (tried, reverted),
  - o_proj stays bf16 (no averaging after it; fp8 would be ~3.5% err).
Weights are host-prescaled by 32 (W elems ~ N(0, 1/2048) would land in
e4m3's subnormal range); the 1/32 is folded into the RoPE tables (bf16)
and the V-eviction scale.

Engine-placement lessons (measured, not guessed):
  - GpSimd DSP is ~3x slower than DVE per tensor op (1.15us vs ~0.35us
    for a [128,512] fp16 add) and swapping op kinds forces ~7us library
    reloads; it now does ONLY dma_start issue. (Moving rope-adds or
    smacc there cost 50-290us total - reverted.)
  - Softmax sums accumulate on the PE: per-pair ones8 [128,2,128]
    DoubleRow matmuls into a psum whose 128 identical rows make the
    sum pre-broadcast, so the normalize chain is just reciprocal (DVE,
    full height) -> aot multiply. No partition_broadcast at all.
    (DVE smacc adds convoy the mask->PV chain: +80us - reverted.)
  - fp8 stale-byte hazard: pt8 ring slots hold old fp16 bytes that
    alias to e4m3 NaN, and NaN*0=NaN, so diagonal-pair gaps are
    memset to 0 instead of relying on the mask multiply.
  - Phase P runs fp8-first: the first V matmuls gate on ~0.5MB of fp8
    stream instead of 2.6MB of bf16 under the ~20us DMA bandwidth
    ramp; bf16 streams (xc0, wv, wq/wk) land during fp8 compute.
  - Q and K are projected per head, head 0 first, so attention starts
    while later heads' RoPE evictions drain the DVE queue.
  - Do NOT emit chunk 2's o_proj inside chunk 3's head loop: +55us
    (tried, reverted; cause unclear - keep emission after the loop).

Carried over from v1: SBUF-resident per-head QT/KT, 4-deep weight tile
rings, staged per-head normalize chain (one stage pumped per tile of
the following head), o_proj deferred one chunk and its bf16 partials
spread over 3 DMA queues.
"""
import os
import sys

if "/opt/trn_rl_repo" not in sys.path:
    sys.path.insert(0, "/opt/trn_rl_repo")

import numpy as np
import ml_dtypes

import concourse.bass as bass
import concourse.mybir as mybir
import concourse.tile as tile
from concourse import bacc
from concourse.bass_utils import run_bass_kernel_spmd
from contextlib import ExitStack

P = 128
B, S, HID, NH = 2, 2048, 2048, 16
HD = HID // NH              # 128
H = 4                       # heads per core
DPC = H * HD                # 512 dims per core
KO = HID // P               # 16 contraction chunks
SC = S // 512               # 4 seq chunks of 512
ST = S // P                 # 16 seq tiles of 128
SCALE = 1.0 / float(np.sqrt(HD))
WS = 32.0                   # host weight prescale (fp8 range)
PB = -2.0                   # exp bias for fp8 P (cancels in softmax)

f32 = mybir.dt.float32
bf16 = mybir.dt.bfloat16
fp16 = mybir.dt.float16
fp8 = mybir.dt.float8e4
DR = mybir.MatmulPerfMode.DoubleRow

_CACHED_NC = None


def build_nc():
    DBG = os.environ.get("KDBG", "0") == "1"
    AF = mybir.ActivationFunctionType
    nc = bacc.Bacc(None, target_bir_lowering=False)

    # chunk 0 of x in bf16 (early-query exact path); chunks 1-3 fp8
    xt = nc.declare_dram_parameter("xt", [P, KO, 256], bf16, isOutput=False)
    xt8 = nc.declare_dram_parameter("xt8", [P, KO, 1792], fp8, isOutput=False)
    wq = nc.declare_dram_parameter("wq", [H, P, KO, HD], bf16, isOutput=False)
    wk = nc.declare_dram_parameter("wk", [H, P, KO, HD], bf16, isOutput=False)
    wq8 = nc.declare_dram_parameter("wq8", [H, P, KO, HD], fp8, isOutput=False)
    wk8 = nc.declare_dram_parameter("wk8", [H, P, KO, HD], fp8, isOutput=False)
    wv = nc.declare_dram_parameter("wv", [P, KO, DPC], bf16, isOutput=False)
    wv8 = nc.declare_dram_parameter("wv8", [P, KO, DPC], fp8, isOutput=False)
    wo = nc.declare_dram_parameter("wo", [P, H, HID], bf16, isOutput=False)
    cosf = nc.declare_dram_parameter("cosf", [P, S], bf16, isOutput=False)
    sinf = nc.declare_dram_parameter("sinf", [P, S], bf16, isOutput=False)
    bmask = nc.declare_dram_parameter("bmask", [P, H, 512], fp8, isOutput=False)
    # bf16 partials: host sums the 4 head-group partials in f32
    out_p = nc.declare_dram_parameter("out_p", [S, HID], bf16, isOutput=True)
    if DBG:
        dbg_qt = nc.declare_dram_parameter("dbg_qt", [P, S], bf16, isOutput=True)
        dbg_kt = nc.declare_dram_parameter("dbg_kt", [P, S], bf16, isOutput=True)
        dbg_vsb = nc.declare_dram_parameter("dbg_vsb", [P, ST, H, 128], fp8,
                                            isOutput=True)
        dbg_pt = nc.declare_dram_parameter("dbg_pt", [P, 2, 512], fp8,
                                           isOutput=True)
        dbg_sm = nc.declare_dram_parameter("dbg_sm", [1, 512], f32, isOutput=True)
        dbg_aot = nc.declare_dram_parameter("dbg_aot", [SC, P, H, 512], bf16,
                                            isOutput=True)

    out3 = out_p.rearrange("(st p) n -> p st n", p=P)

    with tile.TileContext(nc) as tc:
        with ExitStack() as top:
            vpool = top.enter_context(tc.tile_pool(name="vpool", bufs=1))
            qkres = top.enter_context(tc.tile_pool(name="qkres", bufs=1))
            const = top.enter_context(tc.tile_pool(name="const", bufs=1))

            # fp8 V for all 16 tiles (fp8 P@V, chunks 1-3) + fp16 copy of
            # tiles 0-3 for chunk 0's bf16 path
            vsb = vpool.tile([P, ST, H, 128], fp8)
            vsb0 = vpool.tile([P, 2, H, 128], fp16)
            qt_h = [qkres.tile([P, S], bf16, tag=f"qt{h}", name=f"qt{h}")
                    for h in range(H)]
            kt_h = [qkres.tile([P, S], bf16, tag=f"kt{h}", name=f"kt{h}")
                    for h in range(H)]

            zb = const.tile([P, 1], f32)
            nc.vector.memset(zb[:], 0.0)
            nb = const.tile([P, 1], f32)
            nc.vector.memset(nb[:], PB)
            # warm the scalar-engine exp table so the first attention tile
            # doesn't eat the ACT_TABLE_LOAD latency
            warm = const.tile([P, 1], fp16)
            nc.scalar.activation(warm[:], zb[:], AF.Exp, bias=zb[:], scale=1.0)
            bmt = const.tile([P, H, 512], fp8)

            # ---------------- Phase P: projections ----------------
            # fp8 work leads (first matmul gated on ~0.5MB of fp8 stream vs
            # 2.6MB of bf16 under the DMA bandwidth ramp); the bf16 streams
            # land while the fp8 projections compute.
            with ExitStack() as ctx:
                xpool = ctx.enter_context(tc.tile_pool(name="xp", bufs=1))
                wvpool = ctx.enter_context(tc.tile_pool(name="wvp", bufs=1))
                pp = ctx.enter_context(tc.tile_pool(name="pp", bufs=4, space="PSUM"))

                wvq = [wvpool.tile([P, KO // 4, DPC], bf16, tag=f"wv{j}",
                                   name=f"wv{j}") for j in range(4)]
                # fp8 streams: x chunk 1 in quarter tiles (fine-grained DMA
                # watermarks under the ramp), chunks 2-3 whole; wv8 quarters
                # (pair-aligned)
                x8a = xpool.tile([P, KO, 256], fp8, tag="x8a", name="x8a")
                xq1 = [xpool.tile([P, KO // 4, 512], fp8, tag=f"xq1{j}",
                                  name=f"xq1{j}") for j in range(4)]
                xs8 = [None, xpool.tile([P, KO, 512], fp8, tag="x82", name="x82"),
                       xpool.tile([P, KO, 512], fp8, tag="x83", name="x83")]
                wv8q = [wvpool.tile([P, KO // 4, DPC], fp8, tag=f"w8{j}",
                                    name=f"w8{j}") for j in range(4)]
                cspool = ctx.enter_context(tc.tile_pool(name="cs", bufs=1))
                rtmp = ctx.enter_context(tc.tile_pool(name="rt", bufs=3))
                wpool = ctx.enter_context(tc.tile_pool(name="wqk", bufs=4))
                w8pool = ctx.enter_context(tc.tile_pool(name="wqk8", bufs=4))
                cosT = cspool.tile([P, S], bf16)
                sinT = cspool.tile([P, S], bf16)
                xc0 = xpool.tile([P, KO, 256], bf16, tag="xc0", name="xc0")

                # fp8 moving/stationary slice helpers: kp indexes ko-pairs
                def xq1_pair(kp, so=None):
                    t = xq1[kp // 2][:, 2 * (kp % 2):2 * (kp % 2) + 2]
                    return t if so is None else t[:, :, so:so + P]

                def x8_pair(sc, kp, so=None):
                    if sc == 0:
                        t = x8a[:, 2 * kp:2 * kp + 2]
                    elif sc == 1:
                        return xq1_pair(kp, so)
                    else:
                        t = xs8[sc - 1][:, 2 * kp:2 * kp + 2]
                    return t if so is None else t[:, :, so:so + P]

                # critical fp8 set over all three queues
                nc.sync.dma_start(wv8q[0][:], wv8[:, 0:4])
                nc.scalar.dma_start(xq1[0][:], xt8[:, 0:4, 256:768])
                nc.gpsimd.dma_start(wv8q[1][:], wv8[:, 4:8])
                nc.sync.dma_start(xq1[1][:], xt8[:, 4:8, 256:768])
                nc.scalar.dma_start(wv8q[2][:], wv8[:, 8:12])
                nc.gpsimd.dma_start(xq1[2][:], xt8[:, 8:12, 256:768])
                nc.sync.dma_start(wv8q[3][:], wv8[:, 12:16])
                nc.scalar.dma_start(xq1[3][:], xt8[:, 12:16, 256:768])
                nc.gpsimd.dma_start(x8a[:, 0:8], xt8[:, 0:8, 0:256])
                nc.sync.dma_start(x8a[:, 8:16], xt8[:, 8:16, 0:256])

                # V tiles: fp8 DoubleRow (x stationary pair, wv moving)
                def v_block8(sc, sts=None):
                    for st in (sts if sts is not None else range(sc * 4, sc * 4 + 4)):
                        so = (st % 4) * P if sc != 0 else (st - 2) * P
                        ps = pp.tile([P, 512], f32, tag="vproj", name="ps")
                        for kp in range(KO // 2):
                            nc.tensor.matmul(
                                ps[:],
                                x8_pair(sc, kp, so),
                                wv8q[kp // 2][:, 2 * (kp % 2):2 * (kp % 2) + 2],
                                start=(kp == 0),
                                stop=(kp == KO // 2 - 1),
                                perf_mode=DR,
                            )
                        nc.vector.tensor_scalar_mul(
                            vsb[:, st].rearrange("p h d -> p (h d)"),
                            ps[:], 1.0 / WS,
                        )

                v_block8(1)
                v_block8(0, sts=(2, 3))
                nc.sync.dma_start(xs8[1][:, 0:8], xt8[:, 0:8, 768:1280])
                nc.scalar.dma_start(xs8[1][:, 8:16], xt8[:, 8:16, 768:1280])
                nc.gpsimd.dma_start(cosT[:], cosf[:])
                v_block8(2)
                nc.sync.dma_start(xs8[2][:, 0:8], xt8[:, 0:8, 1280:1792])
                nc.scalar.dma_start(xs8[2][:, 8:16], xt8[:, 8:16, 1280:1792])
                nc.gpsimd.dma_start(sinT[:], sinf[:])
                # bf16 streams for the early-query exact path
                nc.sync.dma_start(wvq[0][:], wv[:, 0:4])
                nc.scalar.dma_start(wvq[1][:], wv[:, 4:8])
                nc.gpsimd.dma_start(xc0[:, 0:8], xt[:, 0:8, :])

                v_block8(3)
                nc.sync.dma_start(wvq[2][:], wv[:, 8:12])
                nc.scalar.dma_start(xc0[:, 8:16], xt[:, 8:16, :])
                nc.gpsimd.dma_start(wvq[3][:], wv[:, 12:16])
                nc.gpsimd.dma_start(bmt[:], bmask[:])

                # V tiles 0-1: bf16 from x cols 0:256
                for st in range(2):
                    so = st * P
                    ps = pp.tile([P, 512], f32, tag="vproj", name="ps")
                    for ko in range(KO):
                        nc.tensor.matmul(
                            ps[:],
                            xc0[:, ko, so:so + P],
                            wvq[ko // 4][:, ko % 4],
                            start=(ko == 0),
                            stop=(ko == KO - 1),
                        )
                    # chunk-0 V: fp16 copy (true scale) + fp8 copy, both DVE
                    nc.vector.tensor_scalar_mul(
                        vsb0[:, st].rearrange("p h d -> p (h d)"),
                        ps[:], 1.0 / WS,
                    )
                    nc.vector.tensor_scalar_mul(
                        vsb[:, st].rearrange("p h d -> p (h d)"),
                        ps[:], 1.0 / WS,
                    )

                def rope_evict(ps, dst, ssl, W=512):
                    # RoPE eviction. The rotate's partition swap runs on the
                    # scalar engine (idle during phase P) as two shifted
                    # PSUM->fp16 copies, so the DVE does one full-height fp16
                    # mul instead of two half-height f32-source muls; DVE
                    # per-chunk time drops below the PE's, which was the QK
                    # phase pacer.
                    sw = rtmp.tile([P, 512], fp16, tag="sw")
                    t0 = rtmp.tile([P, 512], fp16, tag="t0")
                    t1 = rtmp.tile([P, 512], fp16, tag="t1")
                    nc.scalar.activation(sw[0:64, 0:W], ps[64:128, 0:W],
                                         AF.Copy)
                    nc.scalar.activation(sw[64:128, 0:W], ps[0:64, 0:W],
                                         AF.Copy)
                    nc.vector.tensor_mul(t0[:, 0:W], sw[:, 0:W], sinT[:, ssl])
                    nc.vector.tensor_mul(t1[:, 0:W], ps[:, 0:W], cosT[:, ssl])
                    nc.vector.tensor_add(dst[:, ssl], t1[:, 0:W], t0[:, 0:W])

                # Q and K per head, head 0 first: attention (c,h) gates on
                # the per-head QT/KT tile's LAST write, so finishing whole
                # heads early lets the attention stream start while later
                # heads' RoPE evictions drain through the DVE queue.
                for h in range(H):
                    for w4, w84, dst_h in ((wq, wq8, qt_h), (wk, wk8, kt_h)):
                        w8t = w8pool.tile([P, KO, HD], fp8, tag="w8")
                        nc.gpsimd.dma_start(w8t[:], w84[h])
                        wt = wpool.tile([P, KO, HD], bf16, tag="w")
                        nc.scalar.dma_start(wt[:], w4[h])
                        # chunks 1-3 + cols 256:512 of chunk 0: fp8 DoubleRow
                        for sc in range(1, SC):
                            ssl = slice(sc * 512, (sc + 1) * 512)
                            ps = pp.tile([P, 512], f32, tag="proj")
                            for kp in range(KO // 2):
                                nc.tensor.matmul(
                                    ps[:],
                                    w8t[:, 2 * kp:2 * kp + 2],
                                    x8_pair(sc, kp),
                                    start=(kp == 0),
                                    stop=(kp == KO // 2 - 1),
                                    perf_mode=DR,
                                )
                            rope_evict(ps, dst_h[h], ssl)
                        ps = pp.tile([P, 512], f32, tag="proj")
                        for kp in range(KO // 2):
                            nc.tensor.matmul(
                                ps[:, 0:256],
                                w8t[:, 2 * kp:2 * kp + 2],
                                x8_pair(0, kp),
                                start=(kp == 0),
                                stop=(kp == KO // 2 - 1),
                                perf_mode=DR,
                            )
                        rope_evict(ps, dst_h[h], slice(256, 512), W=256)
                        # cols 0:256: bf16 exact
                        ps = pp.tile([P, 512], f32, tag="proj")
                        for ko in range(KO):
                            nc.tensor.matmul(
                                ps[:, 0:256],
                                wt[:, ko],
                                xc0[:, ko],
                                start=(ko == 0),
                                stop=(ko == KO - 1),
                            )
                        rope_evict(ps, dst_h[h], slice(0, 256), W=256)

            if DBG:
                nc.sync.dma_start(dbg_qt[:], qt_h[0][:])
                nc.sync.dma_start(dbg_kt[:], kt_h[0][:])
                nc.sync.dma_start(dbg_vsb[:], vsb[:])

            # ------------- Phase A: attention + interleaved o_proj -------------
            with ExitStack() as ctx:
                ppool = ctx.enter_context(tc.tile_pool(name="ppool", bufs=6))
                smpool = ctx.enter_context(tc.tile_pool(name="smp", bufs=2))
                stage = ctx.enter_context(tc.tile_pool(name="stage", bufs=4))
                aopool = ctx.enter_context(tc.tile_pool(name="ao", bufs=1))
                wopool = ctx.enter_context(tc.tile_pool(name="wop", bufs=1))
                ost = ctx.enter_context(tc.tile_pool(name="ost", bufs=4))
                spsum = ctx.enter_context(tc.tile_pool(name="sps", bufs=2, space="PSUM"))
                opsum = ctx.enter_context(tc.tile_pool(name="ops", bufs=2, space="PSUM"))
                smq = ctx.enter_context(tc.tile_pool(name="smq", bufs=2, space="PSUM"))
                opo = ctx.enter_context(tc.tile_pool(name="opo", bufs=2, space="PSUM"))

                # full-width ones: the softmax-sum matmuls write the sum
                # broadcast across all 128 psum partitions, so the normalize
                # chain needs no gpsimd partition_broadcast (slow DSP, and
                # mixing op kinds forces ~7us library swaps)
                ones_col = const.tile([P, P], fp16)
                nc.vector.memset(ones_col[:], 1.0)
                ones8 = const.tile([P, 2, P], fp8)
                nc.vector.memset(ones8[:], 1.0)
                # wot's dma is issued after chunk 0's first head (it would
                # gate c0's first matmuls via the queue watermark otherwise)
                wot = wopool.tile([P, H, HID], bf16)

                aot_c = [
                    aopool.tile([P, H, 512], bf16, tag=f"aot{c}", name=f"aot{c}")
                    for c in range(SC)
                ]

                def emit_oproj(cc):
                    for st4 in range(4):
                        st = cc * 4 + st4
                        for nch in range(4):
                            g = st4 * 4 + nch
                            pso = opo.tile([P, 512], f32, tag="po", name="pso")
                            for dc in range(H):
                                nc.tensor.matmul(
                                    pso[:],
                                    aot_c[cc][:, dc, st4 * P:(st4 + 1) * P],
                                    wot[:, dc, nch * 512:(nch + 1) * 512],
                                    start=(dc == 0),
                                    stop=(dc == H - 1),
                                )
                            # PSUM->SBUF eviction split between scalar ACT
                            # and DVE (gpsimd cannot read PSUM); bf16 out
                            # halves the write stream, spread over 3 queues
                            ob = ost.tile([P, 512], bf16, tag="ob", name="ob")
                            if g % 2 == 0:
                                nc.scalar.activation(ob[:], pso[:], AF.Copy)
                            else:
                                nc.vector.tensor_copy(ob[:], pso[:])
                            eng = (nc.sync, nc.gpsimd, nc.scalar)[g % 3]
                            eng.dma_start(
                                out3[:, st, nch * 512:(nch + 1) * 512], ob[:]
                            )

                # Per-head normalize chain (sm matmul -> rcp -> gpsimd
                # broadcast -> DVE mul), pumped ONE STAGE PER TILE of the
                # following head(s). Emitting the whole chain at once parks
                # ops at the head of the in-order DVE/tensor queues waiting
                # on cross-engine inputs and convoys the tile stream; staged,
                # every op is data-ready when its queue reaches it.
                pending = []

                def norm_pump():
                    if not pending:
                        return
                    e = pending[0]
                    s = e["s"]
                    e["s"] += 1
                    w = e["w"]
                    if s == 0:
                        # fp8 chunks accumulate the softmax sum on the PE
                        # (ones8 DoubleRow per pair) so e["smps"] is already
                        # set; bf16 parts reduce the DVE-accumulated smacc
                        if e["smps"] is None:
                            e["smps"] = smq.tile([P, 512], f32, tag="smp",
                                                 name="smp")
                            nc.tensor.matmul(
                                e["smps"][:, 0:w], ones_col[:],
                                e["sm"][:, 0:w],
                                start=True, stop=True,
                            )
                    elif s == 1:
                        e["rcp"] = stage.tile([P, 512], f32, tag="rcp",
                                              name="rcp")
                        nc.vector.reciprocal_approx_fast(
                            e["rcp"][:, 0:w], e["smps"][:, 0:w])
                    else:
                        qo = e["qo"]
                        nc.vector.tensor_mul(
                            aot_c[e["c"]][:, e["h"], qo:qo + w],
                            e["ob"][:, 0:w], e["rcp"][:, 0:w])
                        pending.pop(0)

                for c in range(SC):
                    qsl = lambda off: slice(c * 512 + off, (c + 1) * 512)
                    nt = 4 * (c + 1)
                    for h in range(H):
                        # finish the chain that owns the recycled ring slot
                        # before reallocating it (only bites in c0's short
                        # 4-tile heads)
                        while len(pending) >= 2:
                            norm_pump()
                        # attn_outT accumulator [d, sq]
                        ob_ps = opsum.tile([P, 512], f32, tag="obp", name="obp")
                        if c == 0:
                            # chunk 0 part A (queries 0:255): bf16/fp16 exact
                            # path (early-query accuracy: fp8 noise doesn't
                            # average over few keys)
                            smacc = smpool.tile([P, 512], fp16, tag="sma",
                                                name="sma")
                            for ti, t in enumerate(range(2)):
                                norm_pump()
                                off = P * t
                                ps = spsum.tile([P, 512], f32, tag="s")
                                nc.tensor.matmul(
                                    ps[:, off:256],
                                    kt_h[h][:, t * P:(t + 1) * P],
                                    qt_h[h][:, off:256],
                                    start=True,
                                    stop=True,
                                )
                                pt = ppool.tile([P, 512], fp16, tag="pt")
                                nc.scalar.activation(
                                    pt[:, off:256], ps[:, off:256], AF.Exp,
                                    bias=zb[:], scale=SCALE,
                                )
                                nc.vector.tensor_mul(
                                    pt[:, off:256], pt[:, off:256],
                                    bmt[:, t, off:256]
                                )
                                nc.tensor.matmul(
                                    ob_ps[:, off:256],
                                    vsb0[:, t, h],
                                    pt[:, off:256],
                                    start=(ti == 0),
                                    stop=(ti == 1),
                                )
                                if ti == 0:
                                    nc.vector.tensor_copy(
                                        smacc[:, 0:256], pt[:, 0:256])
                                else:
                                    nc.vector.tensor_add(
                                        smacc[:, off:256], smacc[:, off:256],
                                        pt[:, off:256],
                                    )
                            pending.append(
                                {"s": 0, "c": 0, "h": h, "ob": ob_ps,
                                 "sm": smacc, "smps": None, "w": 256,
                                 "qo": 0}
                            )
                            # part B (queries 256:511): fp8 pairs over key
                            # tiles (2,3) diagonal then (0,1)
                            while len(pending) >= 2:
                                norm_pump()
                            ob_ps = opsum.tile([P, 512], f32, tag="obp",
                                               name="obp")
                            smq_ps = smq.tile([P, 512], f32, tag="smp",
                                              name="smq_ps")
                            for pi, (ta, tb) in enumerate(((2, 3), (0, 1))):
                                pt8 = ppool.tile([P, 2, 512], fp8, tag="pt")
                                offs = []
                                for i, t in enumerate((ta, tb)):
                                    norm_pump()
                                    r = t - 2
                                    off = P * max(r, 0)
                                    offs.append(off)
                                    ps = spsum.tile([P, 512], f32, tag="s")
                                    nc.tensor.matmul(
                                        ps[:, off:256],
                                        kt_h[h][:, t * P:(t + 1) * P],
                                        qt_h[h][:, 256 + off:512],
                                        start=True,
                                        stop=True,
                                    )
                                    nc.scalar.activation(
                                        pt8[:, i, off:256], ps[:, off:256],
                                        AF.Exp, bias=nb[:], scale=SCALE,
                                    )
                                    if r >= 0:
                                        if off > 0:
                                            nc.vector.memset(
                                                pt8[:, i, 0:off], 0.0)
                                        nc.vector.tensor_mul(
                                            pt8[:, i, off:256],
                                            pt8[:, i, off:256],
                                            bmt[:, r, off:256],
                                        )
                                nc.tensor.matmul(
                                    ob_ps[:, 0:256],
                                    vsb[:, ta:ta + 2, h],
                                    pt8[:, :, 0:256],
                                    start=(pi == 0),
                                    stop=(pi == 1),
                                    perf_mode=DR,
                                )
                                nc.tensor.matmul(
                                    smq_ps[:, 0:256],
                                    ones8[:],
                                    pt8[:, :, 0:256],
                                    start=(pi == 0),
                                    stop=(pi == 1),
                                    perf_mode=DR,
                                )
                            pending.append(
                                {"s": 0, "c": 0, "h": h, "ob": ob_ps,
                                 "sm": None, "smps": smq_ps, "w": 256,
                                 "qo": 256}
                            )
                        else:
                            # chunks 1-3: fp8 P@V in DoubleRow pairs of
                            # adjacent key tiles; probabilities exp(s-2) in
                            # e4m3, causal mask on DVE, softmax sums via
                            # ones8 DoubleRow into a [16,512] psum (row 0).
                            # diagonal pairs first: their exp+mask latency
                            # hides behind the dense unmasked tail of this
                            # head and the previous head's stream
                            smq_ps = smq.tile([P, 512], f32, tag="smp",
                                              name="smq_ps")
                            t_order = list(range(4 * c, nt)) + list(range(0, 4 * c))
                            npair = nt // 2
                            for pi in range(npair):
                                ta, tb = t_order[2 * pi], t_order[2 * pi + 1]
                                pt8 = ppool.tile([P, 2, 512], fp8, tag="pt")
                                offs = []
                                for i, t in enumerate((ta, tb)):
                                    norm_pump()
                                    if c == SC - 1 and h >= 2:
                                        norm_pump()
                                    r = t - 4 * c
                                    off = P * max(r, 0)
                                    offs.append(off)
                                    ps = spsum.tile([P, 512], f32, tag="s")
                                    nc.tensor.matmul(
                                        ps[:, off:512],
                                        kt_h[h][:, t * P:(t + 1) * P],
                                        qt_h[h][:, qsl(off)],
                                        start=True,
                                        stop=True,
                                    )
                                    nc.scalar.activation(
                                        pt8[:, i, off:512], ps[:, off:512],
                                        AF.Exp, bias=nb[:], scale=SCALE,
                                    )
                                    if r >= 0:
                                        # slot 1 of a diagonal pair: the PV
                                        # matmul reads from the pair's base
                                        # offset, so zero the stale region
                                        # below this slot's diagonal (mask
                                        # multiply can't: stale fp8 bytes can
                                        # be NaN encodings and NaN*0=NaN)
                                        offp = P * 2 * pi
                                        if off > offp:
                                            nc.vector.memset(
                                                pt8[:, i, offp:off], 0.0)
                                        nc.vector.tensor_mul(
                                            pt8[:, i, off:512],
                                            pt8[:, i, off:512],
                                            bmt[:, r, off:512],
                                        )
                                off0 = min(offs)
                                nc.tensor.matmul(
                                    ob_ps[:, off0:512],
                                    vsb[:, ta:ta + 2, h],
                                    pt8[:, :, off0:512],
                                    start=(pi == 0),
                                    stop=(pi == npair - 1),
                                    perf_mode=DR,
                                )
                                nc.tensor.matmul(
                                    smq_ps[:, off0:512],
                                    ones8[:],
                                    pt8[:, :, off0:512],
                                    start=(pi == 0),
                                    stop=(pi == npair - 1),
                                    perf_mode=DR,
                                )
                                if DBG and c == 1 and h == 0 and pi == 0:
                                    nc.sync.dma_start(dbg_pt[:], pt8[:])
                            if DBG and c == 1 and h == 0:
                                sdump = stage.tile([1, 512], f32, tag="sdmp",
                                                   name="sdump")
                                nc.vector.tensor_copy(sdump[:], smq_ps[0:1])
                                nc.sync.dma_start(dbg_sm[:], sdump[:])
                        if c > 0:
                            pending.append(
                                {"s": 0, "c": c, "h": h, "ob": ob_ps,
                                 "sm": None, "smps": smq_ps, "w": 512,
                                 "qo": 0}
                            )
                        if c == 0 and h == 0:
                            # issue late so it doesn't gate c0's matmuls
                            nc.gpsimd.dma_start(wot[:], wo[:])

                    # o_proj deferred by one chunk: its aot inputs are then
                    # guaranteed ready, so the PE stream never stalls on the
                    # normalize tail
                    if c > 0:
                        emit_oproj(c - 1)
                while pending:
                    norm_pump()
                emit_oproj(SC - 1)
                if DBG:
                    for c in range(SC):
                        nc.sync.dma_start(dbg_aot[c], aot_c[c][:])

    nc.compile()
    return nc


def _host_prep(hidden_states, position_ids, Wq, Wk, Wv, Wo):
    """Build the 8 per-core input maps (bf16/fp8 weights/activations)."""
    inv_freq = 1.0 / (10000.0 ** (np.arange(0, HD, 2, dtype=np.float32) / HD))
    t = np.arange(S, dtype=np.float32)
    freqs = np.outer(t, inv_freq).astype(np.float32)  # [S, 64]

    bm = np.empty((P, H, 512), dtype=np.float32)
    i = np.arange(P)[:, None, None]
    r = np.arange(H)[None, :, None]
    j = np.arange(512)[None, None, :]
    bm[:] = np.where(i + P * r <= j, 1.0, 0.0)
    bm = bm.astype(ml_dtypes.float8_e4m3)

    in_maps = []
    per_batch = []
    for b in range(B):
        xT = np.ascontiguousarray(hidden_states[b].T)  # [HID, S]
        xt_sw = np.ascontiguousarray(
            xT.reshape(KO, P, S).transpose(1, 0, 2)
        )  # [P, KO, S] f32
        xt_b = np.ascontiguousarray(xt_sw[:, :, 0:256]).astype(
            ml_dtypes.bfloat16)
        xt_8 = np.ascontiguousarray(xt_sw[:, :, 256:2048]).astype(
            ml_dtypes.float8_e4m3)
        fp = freqs[position_ids[b]]  # [S, 64]
        ch = np.cos(fp).T / WS       # [64, S]; 1/32 weight descale folded in
        sh = np.sin(fp).T / WS
        cosf = np.ascontiguousarray(np.concatenate([ch, ch], axis=0)).astype(
            ml_dtypes.bfloat16)   # [128, S]
        sinf = np.ascontiguousarray(np.concatenate([-sh, sh], axis=0)).astype(
            ml_dtypes.bfloat16)  # signed
        per_batch.append((xt_b, xt_8, cosf, sinf))

    for core in range(8):
        b, hg = core // 4, core % 4
        sl = slice(hg * DPC, (hg + 1) * DPC)
        xt_b, xt_8, cosf, sinf = per_batch[b]
        wq_sw = np.ascontiguousarray(
            Wq[sl].T.reshape(KO, P, H, HD).transpose(2, 1, 0, 3)) * WS
        wk_sw = np.ascontiguousarray(
            Wk[sl].T.reshape(KO, P, H, HD).transpose(2, 1, 0, 3)) * WS
        wv_sw = np.ascontiguousarray(
            Wv[sl].T.reshape(KO, P, DPC).transpose(1, 0, 2)) * WS
        wo_sw = np.ascontiguousarray(
            Wo[:, sl].T.reshape(H, HD, HID).transpose(1, 0, 2)
        ).astype(ml_dtypes.bfloat16)  # [P, H, HID]
        in_maps.append({
            "xt": xt_b, "xt8": xt_8,
            "wq": wq_sw.astype(ml_dtypes.bfloat16),
            "wk": wk_sw.astype(ml_dtypes.bfloat16),
            "wq8": wq_sw.astype(ml_dtypes.float8_e4m3),
            "wk8": wk_sw.astype(ml_dtypes.float8_e4m3),
            "wv": wv_sw.astype(ml_dtypes.bfloat16),
            "wv8": wv_sw.astype(ml_dtypes.float8_e4m3),
            "wo": wo_sw,
            "cosf": cosf, "sinf": sinf, "bmask": bm,
        })
    return in_maps


def kernel(hidden_states, attention_mask, position_ids, Wq, Wk, Wv, Wo,
           _trace=False, _trace_kwargs=None):
    global _CACHED_NC
    hidden_states = np.asarray(hidden_states, dtype=np.float32)
    position_ids = np.asarray(position_ids)
    Wq, Wk, Wv, Wo = (np.asarray(w, dtype=np.float32) for w in (Wq, Wk, Wv, Wo))

    if _CACHED_NC is None:
        _CACHED_NC = build_nc()
    nc = _CACHED_NC

    in_maps = _host_prep(hidden_states, position_ids, Wq, Wk, Wv, Wo)
    res = run_bass_kernel_spmd(
        nc, in_maps, list(range(8)), trace=_trace, **(_trace_kwargs or {})
    )

    out = np.empty((B, S, HID), dtype=np.float32)
    for b in range(B):
        acc = res.results[b * 4]["out_p"].astype(np.float32)
        for hg in range(1, 4):
            acc = acc + res.results[b * 4 + hg]["out_p"].astype(np.float32)
        out[b] = acc
    if _trace:
        return out, res
    return out


# revision 40
# speedup vs baseline: 1.1668x; 1.0003x over previous
"""TRN2 Bass kernel for causal multi-head attention with RoPE.

Problem: B=2, S=2048, HID=2048, NH=16, HD=128 (fp32 in/out).
Sharding: 8 cores = 2 (batch) x 4 (head-groups of 4 heads).
Each core computes q/k/v projections for its 4 heads (column-parallel),
RoPE, causal attention, and a row-parallel partial o_proj; the host sums
the 4 partials per batch.

v2 (363us -> ~287us): fp8 (e4m3) DoubleRow matmuls at 2x bf16 PE rate
for the bulk of the work, exploiting the loose 2e-2 rel-err gate
(final rel err ~8.5e-3). Error analysis: softmax here is broad (logit
std ~= 1), so iid fp8 quantization noise on q/k/v/P averages down by
~1/sqrt(N_keys) for late queries; only EARLY queries (few keys) and
the final o_proj see fp8 noise unattenuated. Hence:
  - queries/keys 0..255 run a bf16/fp16 exact path (bf16 q/k/v
    projections, fp16 probabilities, fp16 V copy),
  - everything else uses fp8 DoubleRow for Q/K/V projections and P@V
    in pairs of adjacent key tiles (P stored fp8 with exp bias -2 so
    values fit e4m3's +-240 range - the bias cancels in softmax
    normalization),
  - scores stay bf16: DoubleRow needs the d=128 contraction split to
    K=64, and K=64 DR measures HALF the K=128 rate (417 vs 211 ns per
    512-col matmul) - zero gain, so don't (tried, reverted),
  - o_proj stays bf16 (no averaging after it; fp8 would be ~3.5% err).
Weights are host-prescaled by 32 (W elems ~ N(0, 1/2048) would land in
e4m3's subnormal range); the 1/32 is folded into the RoPE tables (bf16)
and the V-eviction scale.

Engine-placement lessons (measured, not guessed):
  - GpSimd DSP is ~3x slower than DVE per tensor op (1.15us vs ~0.35us
    for a [128,512] fp16 add) and swapping op kinds forces ~7us library
    reloads; it now does ONLY dma_start issue. (Moving rope-adds or
    smacc there cost 50-290us total - reverted.)
  - Softmax sums accumulate on the PE: per-pair ones8 [128,2,128]
    DoubleRow matmuls into a psum whose 128 identical rows make the
    sum pre-broadcast, so the normalize chain is just reciprocal (DVE,
    full height) -> aot multiply. No partition_broadcast at all.
    (DVE smacc adds convoy the mask->PV chain: +80us - reverted.)
  - fp8 stale-byte hazard: pt8 ring slots hold old fp16 bytes that
    alias to e4m3 NaN, and NaN*0=NaN, so diagonal-pair gaps are
    memset to 0 instead of relying on the mask multiply.
  - Phase P runs fp8-first: the first V matmuls gate on ~0.5MB of fp8
    stream instead of 2.6MB of bf16 under the ~20us DMA bandwidth
    ramp; bf16 streams (xc0, wv, wq/wk) land during fp8 compute.
  - Q and K are projected per head, head 0 first, so attention starts
    while later heads' RoPE evictions drain the DVE queue.
  - Do NOT emit chunk 2's o_proj inside chunk 3's head loop: +55us
    (tried, reverted; cause unclear - keep emission after the loop).

Carried over from v1: SBUF-resident per-head QT/KT, 4-deep weight tile
rings, staged per-head normalize chain (one stage pumped per tile of
the following head), o_proj deferred one chunk and its bf16 partials
spread over 3 DMA queues.
"""
import os
import sys

if "/opt/trn_rl_repo" not in sys.path:
    sys.path.insert(0, "/opt/trn_rl_repo")

import numpy as np
import ml_dtypes

import concourse.bass as bass
import concourse.mybir as mybir
import concourse.tile as tile
from concourse import bacc
from concourse.bass_utils import run_bass_kernel_spmd
from contextlib import ExitStack

P = 128
B, S, HID, NH = 2, 2048, 2048, 16
HD = HID // NH              # 128
H = 4                       # heads per core
DPC = H * HD                # 512 dims per core
KO = HID // P               # 16 contraction chunks
SC = S // 512               # 4 seq chunks of 512
ST = S // P                 # 16 seq tiles of 128
SCALE = 1.0 / float(np.sqrt(HD))
WS = 32.0                   # host weight prescale (fp8 range)
PB = -2.0                   # exp bias for fp8 P (cancels in softmax)

f32 = mybir.dt.float32
bf16 = mybir.dt.bfloat16
fp16 = mybir.dt.float16
fp8 = mybir.dt.float8e4
DR = mybir.MatmulPerfMode.DoubleRow

_CACHED_NC = None


def build_nc():
    DBG = os.environ.get("KDBG", "0") == "1"
    AF = mybir.ActivationFunctionType
    nc = bacc.Bacc(None, target_bir_lowering=False)

    # chunk 0 of x in bf16 (early-query exact path); chunks 1-3 fp8
    xt = nc.declare_dram_parameter("xt", [P, KO, 256], bf16, isOutput=False)
    xt8 = nc.declare_dram_parameter("xt8", [P, KO, 1792], fp8, isOutput=False)
    wq = nc.declare_dram_parameter("wq", [H, P, KO, HD], bf16, isOutput=False)
    wk = nc.declare_dram_parameter("wk", [H, P, KO, HD], bf16, isOutput=False)
    wq8 = nc.declare_dram_parameter("wq8", [H, P, KO, HD], fp8, isOutput=False)
    wk8 = nc.declare_dram_parameter("wk8", [H, P, KO, HD], fp8, isOutput=False)
    wv = nc.declare_dram_parameter("wv", [P, KO, DPC], bf16, isOutput=False)
    wv8 = nc.declare_dram_parameter("wv8", [P, KO, DPC], fp8, isOutput=False)
    wo = nc.declare_dram_parameter("wo", [P, H, HID], bf16, isOutput=False)
    cosf = nc.declare_dram_parameter("cosf", [P, S], bf16, isOutput=False)
    sinf = nc.declare_dram_parameter("sinf", [P, S], bf16, isOutput=False)
    bmask = nc.declare_dram_parameter("bmask", [P, H, 512], fp8, isOutput=False)
    # bf16 partials: host sums the 4 head-group partials in f32
    out_p = nc.declare_dram_parameter("out_p", [S, HID], bf16, isOutput=True)
    if DBG:
        dbg_qt = nc.declare_dram_parameter("dbg_qt", [P, S], bf16, isOutput=True)
        dbg_kt = nc.declare_dram_parameter("dbg_kt", [P, S], bf16, isOutput=True)
        dbg_vsb = nc.declare_dram_parameter("dbg_vsb", [P, ST, H, 128], fp8,
                                            isOutput=True)
        dbg_pt = nc.declare_dram_parameter("dbg_pt", [P, 2, 512], fp8,
                                           isOutput=True)
        dbg_sm = nc.declare_dram_parameter("dbg_sm", [1, 512], f32, isOutput=True)
        dbg_aot = nc.declare_dram_parameter("dbg_aot", [SC, P, H, 512], bf16,
                                            isOutput=True)

    out3 = out_p.rearrange("(st p) n -> p st n", p=P)

    with tile.TileContext(nc) as tc:
        with ExitStack() as top:
            vpool = top.enter_context(tc.tile_pool(name="vpool", bufs=1))
            qkres = top.enter_context(tc.tile_pool(name="qkres", bufs=1))
            const = top.enter_context(tc.tile_pool(name="const", bufs=1))

            # fp8 V for all 16 tiles (fp8 P@V, chunks 1-3) + fp16 copy of
            # tiles 0-3 for chunk 0's bf16 path
            vsb = vpool.tile([P, ST, H, 128], fp8)
            vsb0 = vpool.tile([P, 2, H, 128], fp16)
            qt_h = [qkres.tile([P, S], bf16, tag=f"qt{h}", name=f"qt{h}")
                    for h in range(H)]
            kt_h = [qkres.tile([P, S], bf16, tag=f"kt{h}", name=f"kt{h}")
                    for h in range(H)]

            zb = const.tile([P, 1], f32)
            nc.vector.memset(zb[:], 0.0)
            nb = const.tile([P, 1], f32)
            nc.vector.memset(nb[:], PB)
            # warm the scalar-engine exp table so the first attention tile
            # doesn't eat the ACT_TABLE_LOAD latency
            warm = const.tile([P, 1], fp16)
            nc.scalar.activation(warm[:], zb[:], AF.Exp, bias=zb[:], scale=1.0)
            bmt = const.tile([P, H, 512], fp8)

            # ---------------- Phase P: projections ----------------
            # fp8 work leads (first matmul gated on ~0.5MB of fp8 stream vs
            # 2.6MB of bf16 under the DMA bandwidth ramp); the bf16 streams
            # land while the fp8 projections compute.
            with ExitStack() as ctx:
                xpool = ctx.enter_context(tc.tile_pool(name="xp", bufs=1))
                wvpool = ctx.enter_context(tc.tile_pool(name="wvp", bufs=1))
                pp = ctx.enter_context(tc.tile_pool(name="pp", bufs=4, space="PSUM"))

                wvq = [wvpool.tile([P, KO // 4, DPC], bf16, tag=f"wv{j}",
                                   name=f"wv{j}") for j in range(4)]
                # fp8 streams: x chunk 1 in quarter tiles (fine-grained DMA
                # watermarks under the ramp), chunks 2-3 whole; wv8 quarters
                # (pair-aligned)
                x8a = xpool.tile([P, KO, 256], fp8, tag="x8a", name="x8a")
                xq1 = [xpool.tile([P, KO // 4, 512], fp8, tag=f"xq1{j}",
                                  name=f"xq1{j}") for j in range(4)]
                xs8 = [None, xpool.tile([P, KO, 512], fp8, tag="x82", name="x82"),
                       xpool.tile([P, KO, 512], fp8, tag="x83", name="x83")]
                wv8q = [wvpool.tile([P, KO // 4, DPC], fp8, tag=f"w8{j}",
                                    name=f"w8{j}") for j in range(4)]
                cspool = ctx.enter_context(tc.tile_pool(name="cs", bufs=1))
                rtmp = ctx.enter_context(tc.tile_pool(name="rt", bufs=3))
                wpool = ctx.enter_context(tc.tile_pool(name="wqk", bufs=4))
                w8pool = ctx.enter_context(tc.tile_pool(name="wqk8", bufs=4))
                cosT = cspool.tile([P, S], bf16)
                sinT = cspool.tile([P, S], bf16)
                xc0 = xpool.tile([P, KO, 256], bf16, tag="xc0", name="xc0")

                # fp8 moving/stationary slice helpers: kp indexes ko-pairs
                def xq1_pair(kp, so=None):
                    t = xq1[kp // 2][:, 2 * (kp % 2):2 * (kp % 2) + 2]
                    return t if so is None else t[:, :, so:so + P]

                def x8_pair(sc, kp, so=None):
                    if sc == 0:
                        t = x8a[:, 2 * kp:2 * kp + 2]
                    elif sc == 1:
                        return xq1_pair(kp, so)
                    else:
                        t = xs8[sc - 1][:, 2 * kp:2 * kp + 2]
                    return t if so is None else t[:, :, so:so + P]

                # critical fp8 set over all three queues
                nc.sync.dma_start(wv8q[0][:], wv8[:, 0:4])
                nc.scalar.dma_start(xq1[0][:], xt8[:, 0:4, 256:768])
                nc.gpsimd.dma_start(wv8q[1][:], wv8[:, 4:8])
                nc.sync.dma_start(xq1[1][:], xt8[:, 4:8, 256:768])
                nc.scalar.dma_start(wv8q[2][:], wv8[:, 8:12])
                nc.gpsimd.dma_start(xq1[2][:], xt8[:, 8:12, 256:768])
                nc.sync.dma_start(wv8q[3][:], wv8[:, 12:16])
                nc.scalar.dma_start(xq1[3][:], xt8[:, 12:16, 256:768])
                nc.gpsimd.dma_start(x8a[:, 0:8], xt8[:, 0:8, 0:256])
                nc.sync.dma_start(x8a[:, 8:16], xt8[:, 8:16, 0:256])

                # V tiles: fp8 DoubleRow (x stationary pair, wv moving)
                def v_block8(sc, sts=None):
                    for st in (sts if sts is not None else range(sc * 4, sc * 4 + 4)):
                        so = (st % 4) * P if sc != 0 else (st - 2) * P
                        ps = pp.tile([P, 512], f32, tag="vproj", name="ps")
                        for kp in range(KO // 2):
                            nc.tensor.matmul(
                                ps[:],
                                x8_pair(sc, kp, so),
                                wv8q[kp // 2][:, 2 * (kp % 2):2 * (kp % 2) + 2],
                                start=(kp == 0),
                                stop=(kp == KO // 2 - 1),
                                perf_mode=DR,
                            )
                        nc.vector.tensor_scalar_mul(
                            vsb[:, st].rearrange("p h d -> p (h d)"),
                            ps[:], 1.0 / WS,
                        )

                v_block8(1)
                v_block8(0, sts=(2, 3))
                nc.sync.dma_start(xs8[1][:, 0:8], xt8[:, 0:8, 768:1280])
                nc.scalar.dma_start(xs8[1][:, 8:16], xt8[:, 8:16, 768:1280])
                nc.gpsimd.dma_start(cosT[:], cosf[:])
                v_block8(2)
                nc.sync.dma_start(xs8[2][:, 0:8], xt8[:, 0:8, 1280:1792])
                nc.scalar.dma_start(xs8[2][:, 8:16], xt8[:, 8:16, 1280:1792])
                nc.gpsimd.dma_start(sinT[:], sinf[:])
                # bf16 streams for the early-query exact path
                nc.sync.dma_start(wvq[0][:], wv[:, 0:4])
                nc.scalar.dma_start(wvq[1][:], wv[:, 4:8])
                nc.gpsimd.dma_start(xc0[:, 0:8], xt[:, 0:8, :])

                v_block8(3)
                nc.sync.dma_start(wvq[2][:], wv[:, 8:12])
                nc.scalar.dma_start(xc0[:, 8:16], xt[:, 8:16, :])
                nc.gpsimd.dma_start(wvq[3][:], wv[:, 12:16])
                nc.gpsimd.dma_start(bmt[:], bmask[:])

                # V tiles 0-1: bf16 from x cols 0:256
                for st in range(2):
                    so = st * P
                    ps = pp.tile([P, 512], f32, tag="vproj", name="ps")
                    for ko in range(KO):
                        nc.tensor.matmul(
                            ps[:],
                            xc0[:, ko, so:so + P],
                            wvq[ko // 4][:, ko % 4],
                            start=(ko == 0),
                            stop=(ko == KO - 1),
                        )
                    # chunk-0 V: fp16 copy (true scale) + fp8 copy, both DVE
                    nc.vector.tensor_scalar_mul(
                        vsb0[:, st].rearrange("p h d -> p (h d)"),
                        ps[:], 1.0 / WS,
                    )
                    nc.vector.tensor_scalar_mul(
                        vsb[:, st].rearrange("p h d -> p (h d)"),
                        ps[:], 1.0 / WS,
                    )

                def rope_evict(ps, dst, ssl, W=512):
                    # RoPE eviction. The rotate's partition swap runs on the
                    # scalar engine (idle during phase P) as two shifted
                    # PSUM->fp16 copies, so the DVE does one full-height fp16
                    # mul instead of two half-height f32-source muls; DVE
                    # per-chunk time drops below the PE's, which was the QK
                    # phase pacer.
                    sw = rtmp.tile([P, 512], fp16, tag="sw")
                    t0 = rtmp.tile([P, 512], fp16, tag="t0")
                    t1 = rtmp.tile([P, 512], fp16, tag="t1")
                    nc.scalar.activation(sw[0:64, 0:W], ps[64:128, 0:W],
                                         AF.Copy)
                    nc.scalar.activation(sw[64:128, 0:W], ps[0:64, 0:W],
                                         AF.Copy)
                    nc.vector.tensor_mul(t0[:, 0:W], sw[:, 0:W], sinT[:, ssl])
                    nc.vector.tensor_mul(t1[:, 0:W], ps[:, 0:W], cosT[:, ssl])
                    nc.vector.tensor_add(dst[:, ssl], t1[:, 0:W], t0[:, 0:W])

                # Q and K per head, head 0 first: attention (c,h) gates on
                # the per-head QT/KT tile's LAST write, so finishing whole
                # heads early lets the attention stream start while later
                # heads' RoPE evictions drain through the DVE queue.
                for h in range(H):
                    for w4, w84, dst_h in ((wq, wq8, qt_h), (wk, wk8, kt_h)):
                        w8t = w8pool.tile([P, KO, HD], fp8, tag="w8")
                        nc.gpsimd.dma_start(w8t[:], w84[h])
                        wt = wpool.tile([P, KO, HD], bf16, tag="w")
                        nc.scalar.dma_start(wt[:], w4[h])
                        # chunks 1-3 + cols 256:512 of chunk 0: fp8 DoubleRow
                        for sc in range(1, SC):
                            ssl = slice(sc * 512, (sc + 1) * 512)
                            ps = pp.tile([P, 512], f32, tag="proj")
                            for kp in range(KO // 2):
                                nc.tensor.matmul(
                                    ps[:],
                                    w8t[:, 2 * kp:2 * kp + 2],
                                    x8_pair(sc, kp),
                                    start=(kp == 0),
                                    stop=(kp == KO // 2 - 1),
                                    perf_mode=DR,
                                )
                            rope_evict(ps, dst_h[h], ssl)
                        ps = pp.tile([P, 512], f32, tag="proj")
                        for kp in range(KO // 2):
                            nc.tensor.matmul(
                                ps[:, 0:256],
                                w8t[:, 2 * kp:2 * kp + 2],
                                x8_pair(0, kp),
                                start=(kp == 0),
                                stop=(kp == KO // 2 - 1),
                                perf_mode=DR,
                            )
                        rope_evict(ps, dst_h[h], slice(256, 512), W=256)
                        # cols 0:256: bf16 exact
                        ps = pp.tile([P, 512], f32, tag="proj")
                        for ko in range(KO):
                            nc.tensor.matmul(
                                ps[:, 0:256],
                                wt[:, ko],
                                xc0[:, ko],
                                start=(ko == 0),
                                stop=(ko == KO - 1),
                            )
                        rope_evict(ps, dst_h[h], slice(0, 256), W=256)

            if DBG:
                nc.sync.dma_start(dbg_qt[:], qt_h[0][:])
                nc.sync.dma_start(dbg_kt[:], kt_h[0][:])
                nc.sync.dma_start(dbg_vsb[:], vsb[:])

            # ------------- Phase A: attention + interleaved o_proj -------------
            with ExitStack() as ctx:
                ppool = ctx.enter_context(tc.tile_pool(name="ppool", bufs=6))
                smpool = ctx.enter_context(tc.tile_pool(name="smp", bufs=2))
                stage = ctx.enter_context(tc.tile_pool(name="stage", bufs=4))
                aopool = ctx.enter_context(tc.tile_pool(name="ao", bufs=1))
                wopool = ctx.enter_context(tc.tile_pool(name="wop", bufs=1))
                ost = ctx.enter_context(tc.tile_pool(name="ost", bufs=4))
                # attention psum pools live in an inner scope released
                # before the FINAL chunk's o_proj, whose eviction ring can
                # then go 4 deep (ring-2 makes it eviction-paced)
                opo = ctx.enter_context(tc.tile_pool(name="opo", bufs=2, space="PSUM"))
                pctx = ExitStack()
                spsum = pctx.enter_context(tc.tile_pool(name="sps", bufs=2, space="PSUM"))
                opsum = pctx.enter_context(tc.tile_pool(name="ops", bufs=2, space="PSUM"))
                smq = pctx.enter_context(tc.tile_pool(name="smq", bufs=2, space="PSUM"))

                # full-width ones: the softmax-sum matmuls write the sum
                # broadcast across all 128 psum partitions, so the normalize
                # chain needs no gpsimd partition_broadcast (slow DSP, and
                # mixing op kinds forces ~7us library swaps)
                ones_col = const.tile([P, P], fp16)
                nc.vector.memset(ones_col[:], 1.0)
                ones8 = const.tile([P, 2, P], fp8)
                nc.vector.memset(ones8[:], 1.0)
                # wot's dma is issued after chunk 0's first head (it would
                # gate c0's first matmuls via the queue watermark otherwise)
                wot = wopool.tile([P, H, HID], bf16)

                aot_c = [
                    aopool.tile([P, H, 512], bf16, tag=f"aot{c}", name=f"aot{c}")
                    for c in range(SC)
                ]

                def emit_oproj(cc, pool=None, tag="po"):
                    for st4 in range(4):
                        st = cc * 4 + st4
                        for nch in range(4):
                            g = st4 * 4 + nch
                            pso = (pool or opo).tile([P, 512], f32, tag=tag,
                                                     name="pso")
                            for dc in range(H):
                                nc.tensor.matmul(
                                    pso[:],
                                    aot_c[cc][:, dc, st4 * P:(st4 + 1) * P],
                                    wot[:, dc, nch * 512:(nch + 1) * 512],
                                    start=(dc == 0),
                                    stop=(dc == H - 1),
                                )
                            # PSUM->SBUF eviction split between scalar ACT
                            # and DVE (gpsimd cannot read PSUM); bf16 out
                            # halves the write stream, spread over 3 queues
                            ob = ost.tile([P, 512], bf16, tag="ob", name="ob")
                            if g % 2 == 0:
                                nc.scalar.activation(ob[:], pso[:], AF.Copy)
                            else:
                                nc.vector.tensor_copy(ob[:], pso[:])
                            eng = (nc.sync, nc.gpsimd, nc.scalar)[g % 3]
                            eng.dma_start(
                                out3[:, st, nch * 512:(nch + 1) * 512], ob[:]
                            )

                # Per-head normalize chain (sm matmul -> rcp -> gpsimd
                # broadcast -> DVE mul), pumped ONE STAGE PER TILE of the
                # following head(s). Emitting the whole chain at once parks
                # ops at the head of the in-order DVE/tensor queues waiting
                # on cross-engine inputs and convoys the tile stream; staged,
                # every op is data-ready when its queue reaches it.
                pending = []

                def norm_pump():
                    if not pending:
                        return
                    e = pending[0]
                    s = e["s"]
                    e["s"] += 1
                    w = e["w"]
                    if s == 0:
                        # fp8 chunks accumulate the softmax sum on the PE
                        # (ones8 DoubleRow per pair) so e["smps"] is already
                        # set; bf16 parts reduce the DVE-accumulated smacc
                        if e["smps"] is None:
                            e["smps"] = smq.tile([P, 512], f32, tag="smp",
                                                 name="smp")
                            nc.tensor.matmul(
                                e["smps"][:, 0:w], ones_col[:],
                                e["sm"][:, 0:w],
                                start=True, stop=True,
                            )
                    elif s == 1:
                        e["rcp"] = stage.tile([P, 512], f32, tag="rcp",
                                              name="rcp")
                        nc.vector.reciprocal_approx_fast(
                            e["rcp"][:, 0:w], e["smps"][:, 0:w])
                    else:
                        qo = e["qo"]
                        nc.vector.tensor_mul(
                            aot_c[e["c"]][:, e["h"], qo:qo + w],
                            e["ob"][:, 0:w], e["rcp"][:, 0:w])
                        pending.pop(0)

                for c in range(SC):
                    qsl = lambda off: slice(c * 512 + off, (c + 1) * 512)
                    nt = 4 * (c + 1)
                    for h in range(H):
                        # finish the chain that owns the recycled ring slot
                        # before reallocating it (only bites in c0's short
                        # 4-tile heads)
                        while len(pending) >= 2:
                            norm_pump()
                        # attn_outT accumulator [d, sq]
                        ob_ps = opsum.tile([P, 512], f32, tag="obp", name="obp")
                        if c == 0:
                            # chunk 0 part A (queries 0:255): bf16/fp16 exact
                            # path (early-query accuracy: fp8 noise doesn't
                            # average over few keys)
                            smacc = smpool.tile([P, 512], fp16, tag="sma",
                                                name="sma")
                            for ti, t in enumerate(range(2)):
                                norm_pump()
                                off = P * t
                                ps = spsum.tile([P, 512], f32, tag="s")
                                nc.tensor.matmul(
                                    ps[:, off:256],
                                    kt_h[h][:, t * P:(t + 1) * P],
                                    qt_h[h][:, off:256],
                                    start=True,
                                    stop=True,
                                )
                                pt = ppool.tile([P, 512], fp16, tag="pt")
                                nc.scalar.activation(
                                    pt[:, off:256], ps[:, off:256], AF.Exp,
                                    bias=zb[:], scale=SCALE,
                                )
                                nc.vector.tensor_mul(
                                    pt[:, off:256], pt[:, off:256],
                                    bmt[:, t, off:256]
                                )
                                nc.tensor.matmul(
                                    ob_ps[:, off:256],
                                    vsb0[:, t, h],
                                    pt[:, off:256],
                                    start=(ti == 0),
                                    stop=(ti == 1),
                                )
                                if ti == 0:
                                    nc.vector.tensor_copy(
                                        smacc[:, 0:256], pt[:, 0:256])
                                else:
                                    nc.vector.tensor_add(
                                        smacc[:, off:256], smacc[:, off:256],
                                        pt[:, off:256],
                                    )
                            pending.append(
                                {"s": 0, "c": 0, "h": h, "ob": ob_ps,
                                 "sm": smacc, "smps": None, "w": 256,
                                 "qo": 0}
                            )
                            # part B (queries 256:511): fp8 pairs over key
                            # tiles (2,3) diagonal then (0,1)
                            while len(pending) >= 2:
                                norm_pump()
                            ob_ps = opsum.tile([P, 512], f32, tag="obp",
                                               name="obp")
                            smq_ps = smq.tile([P, 512], f32, tag="smp",
                                              name="smq_ps")
                            for pi, (ta, tb) in enumerate(((2, 3), (0, 1))):
                                pt8 = ppool.tile([P, 2, 512], fp8, tag="pt")
                                offs = []
                                for i, t in enumerate((ta, tb)):
                                    norm_pump()
                                    r = t - 2
                                    off = P * max(r, 0)
                                    offs.append(off)
                                    ps = spsum.tile([P, 512], f32, tag="s")
                                    nc.tensor.matmul(
                                        ps[:, off:256],
                                        kt_h[h][:, t * P:(t + 1) * P],
                                        qt_h[h][:, 256 + off:512],
                                        start=True,
                                        stop=True,
                                    )
                                    nc.scalar.activation(
                                        pt8[:, i, off:256], ps[:, off:256],
                                        AF.Exp, bias=nb[:], scale=SCALE,
                                    )
                                    if r >= 0:
                                        if off > 0:
                                            nc.vector.memset(
                                                pt8[:, i, 0:off], 0.0)
                                        nc.vector.tensor_mul(
                                            pt8[:, i, off:256],
                                            pt8[:, i, off:256],
                                            bmt[:, r, off:256],
                                        )
                                nc.tensor.matmul(
                                    ob_ps[:, 0:256],
                                    vsb[:, ta:ta + 2, h],
                                    pt8[:, :, 0:256],
                                    start=(pi == 0),
                                    stop=(pi == 1),
                                    perf_mode=DR,
                                )
                                nc.tensor.matmul(
                                    smq_ps[:, 0:256],
                                    ones8[:],
                                    pt8[:, :, 0:256],
                                    start=(pi == 0),
                                    stop=(pi == 1),
                                    perf_mode=DR,
                                )
                            pending.append(
                                {"s": 0, "c": 0, "h": h, "ob": ob_ps,
                                 "sm": None, "smps": smq_ps, "w": 256,
                                 "qo": 256}
                            )
                        else:
                            # chunks 1-3: fp8 P@V in DoubleRow pairs of
                            # adjacent key tiles; probabilities exp(s-2) in
                            # e4m3, causal mask on DVE, softmax sums via
                            # ones8 DoubleRow into a [16,512] psum (row 0).
                            # diagonal pairs first: their exp+mask latency
                            # hides behind the dense unmasked tail of this
                            # head and the previous head's stream
                            smq_ps = smq.tile([P, 512], f32, tag="smp",
                                              name="smq_ps")
                            t_order = list(range(4 * c, nt)) + list(range(0, 4 * c))
                            npair = nt // 2
                            for pi in range(npair):
                                ta, tb = t_order[2 * pi], t_order[2 * pi + 1]
                                pt8 = ppool.tile([P, 2, 512], fp8, tag="pt")
                                offs = []
                                for i, t in enumerate((ta, tb)):
                                    norm_pump()
                                    if c == SC - 1 and h >= 2:
                                        norm_pump()
                                    r = t - 4 * c
                                    off = P * max(r, 0)
                                    offs.append(off)
                                    ps = spsum.tile([P, 512], f32, tag="s")
                                    nc.tensor.matmul(
                                        ps[:, off:512],
                                        kt_h[h][:, t * P:(t + 1) * P],
                                        qt_h[h][:, qsl(off)],
                                        start=True,
                                        stop=True,
                                    )
                                    nc.scalar.activation(
                                        pt8[:, i, off:512], ps[:, off:512],
                                        AF.Exp, bias=nb[:], scale=SCALE,
                                    )
                                    if r >= 0:
                                        # slot 1 of a diagonal pair: the PV
                                        # matmul reads from the pair's base
                                        # offset, so zero the stale region
                                        # below this slot's diagonal (mask
                                        # multiply can't: stale fp8 bytes can
                                        # be NaN encodings and NaN*0=NaN)
                                        offp = P * 2 * pi
                                        if off > offp:
                                            nc.vector.memset(
                                                pt8[:, i, offp:off], 0.0)
                                        nc.vector.tensor_mul(
                                            pt8[:, i, off:512],
                                            pt8[:, i, off:512],
                                            bmt[:, r, off:512],
                                        )
                                off0 = min(offs)
                                nc.tensor.matmul(
                                    ob_ps[:, off0:512],
                                    vsb[:, ta:ta + 2, h],
                                    pt8[:, :, off0:512],
                                    start=(pi == 0),
                                    stop=(pi == npair - 1),
                                    perf_mode=DR,
                                )
                                nc.tensor.matmul(
                                    smq_ps[:, off0:512],
                                    ones8[:],
                                    pt8[:, :, off0:512],
                                    start=(pi == 0),
                                    stop=(pi == npair - 1),
                                    perf_mode=DR,
                                )
                                if DBG and c == 1 and h == 0 and pi == 0:
                                    nc.sync.dma_start(dbg_pt[:], pt8[:])
                            if DBG and c == 1 and h == 0:
                                sdump = stage.tile([1, 512], f32, tag="sdmp",
                                                   name="sdump")
                                nc.vector.tensor_copy(sdump[:], smq_ps[0:1])
                                nc.sync.dma_start(dbg_sm[:], sdump[:])
                        if c > 0:
                            pending.append(
                                {"s": 0, "c": c, "h": h, "ob": ob_ps,
                                 "sm": None, "smps": smq_ps, "w": 512,
                                 "qo": 0}
                            )
                        if c == 0 and h == 0:
                            # issue late so it doesn't gate c0's matmuls
                            nc.gpsimd.dma_start(wot[:], wo[:])

                    # o_proj deferred by one chunk: its aot inputs are then
                    # guaranteed ready, so the PE stream never stalls on the
                    # normalize tail
                    if c > 0:
                        emit_oproj(c - 1)
                while pending:
                    norm_pump()
                pctx.close()
                opof = ctx.enter_context(
                    tc.tile_pool(name="opof", bufs=4, space="PSUM"))
                emit_oproj(SC - 1, pool=opof, tag="pof")
                if DBG:
                    for c in range(SC):
                        nc.sync.dma_start(dbg_aot[c], aot_c[c][:])

    nc.compile()
    return nc


def _host_prep(hidden_states, position_ids, Wq, Wk, Wv, Wo):
    """Build the 8 per-core input maps (bf16/fp8 weights/activations)."""
    inv_freq = 1.0 / (10000.0 ** (np.arange(0, HD, 2, dtype=np.float32) / HD))
    t = np.arange(S, dtype=np.float32)
    freqs = np.outer(t, inv_freq).astype(np.float32)  # [S, 64]

    bm = np.empty((P, H, 512), dtype=np.float32)
    i = np.arange(P)[:, None, None]
    r = np.arange(H)[None, :, None]
    j = np.arange(512)[None, None, :]
    bm[:] = np.where(i + P * r <= j, 1.0, 0.0)
    bm = bm.astype(ml_dtypes.float8_e4m3)

    in_maps = []
    per_batch = []
    for b in range(B):
        xT = np.ascontiguousarray(hidden_states[b].T)  # [HID, S]
        xt_sw = np.ascontiguousarray(
            xT.reshape(KO, P, S).transpose(1, 0, 2)
        )  # [P, KO, S] f32
        xt_b = np.ascontiguousarray(xt_sw[:, :, 0:256]).astype(
            ml_dtypes.bfloat16)
        xt_8 = np.ascontiguousarray(xt_sw[:, :, 256:2048]).astype(
            ml_dtypes.float8_e4m3)
        fp = freqs[position_ids[b]]  # [S, 64]
        ch = np.cos(fp).T / WS       # [64, S]; 1/32 weight descale folded in
        sh = np.sin(fp).T / WS
        cosf = np.ascontiguousarray(np.concatenate([ch, ch], axis=0)).astype(
            ml_dtypes.bfloat16)   # [128, S]
        sinf = np.ascontiguousarray(np.concatenate([-sh, sh], axis=0)).astype(
            ml_dtypes.bfloat16)  # signed
        per_batch.append((xt_b, xt_8, cosf, sinf))

    for core in range(8):
        b, hg = core // 4, core % 4
        sl = slice(hg * DPC, (hg + 1) * DPC)
        xt_b, xt_8, cosf, sinf = per_batch[b]
        wq_sw = np.ascontiguousarray(
            Wq[sl].T.reshape(KO, P, H, HD).transpose(2, 1, 0, 3)) * WS
        wk_sw = np.ascontiguousarray(
            Wk[sl].T.reshape(KO, P, H, HD).transpose(2, 1, 0, 3)) * WS
        wv_sw = np.ascontiguousarray(
            Wv[sl].T.reshape(KO, P, DPC).transpose(1, 0, 2)) * WS
        wo_sw = np.ascontiguousarray(
            Wo[:, sl].T.reshape(H, HD, HID).transpose(1, 0, 2)
        ).astype(ml_dtypes.bfloat16)  # [P, H, HID]
        in_maps.append({
            "xt": xt_b, "xt8": xt_8,
            "wq": wq_sw.astype(ml_dtypes.bfloat16),
            "wk": wk_sw.astype(ml_dtypes.bfloat16),
            "wq8": wq_sw.astype(ml_dtypes.float8_e4m3),
            "wk8": wk_sw.astype(ml_dtypes.float8_e4m3),
            "wv": wv_sw.astype(ml_dtypes.bfloat16),
            "wv8": wv_sw.astype(ml_dtypes.float8_e4m3),
            "wo": wo_sw,
            "cosf": cosf, "sinf": sinf, "bmask": bm,
        })
    return in_maps


def kernel(hidden_states, attention_mask, position_ids, Wq, Wk, Wv, Wo,
           _trace=False, _trace_kwargs=None):
    global _CACHED_NC
    hidden_states = np.asarray(hidden_states, dtype=np.float32)
    position_ids = np.asarray(position_ids)
    Wq, Wk, Wv, Wo = (np.asarray(w, dtype=np.float32) for w in (Wq, Wk, Wv, Wo))

    if _CACHED_NC is None:
        _CACHED_NC = build_nc()
    nc = _CACHED_NC

    in_maps = _host_prep(hidden_states, position_ids, Wq, Wk, Wv, Wo)
    res = run_bass_kernel_spmd(
        nc, in_maps, list(range(8)), trace=_trace, **(_trace_kwargs or {})
    )

    out = np.empty((B, S, HID), dtype=np.float32)
    for b in range(B):
        acc = res.results[b * 4]["out_p"].astype(np.float32)
        for hg in range(1, 4):
            acc = acc + res.results[b * 4 + hg]["out_p"].astype(np.float32)
        out[b] = acc
    if _trace:
        return out, res
    return out


# revision 41
# speedup vs baseline: 1.1921x; 1.0216x over previous
"""TRN2 Bass kernel for causal multi-head attention with RoPE.

Problem: B=2, S=2048, HID=2048, NH=16, HD=128 (fp32 in/out).
Sharding: 8 cores = 2 (batch) x 4 (head-groups of 4 heads).
Each core computes q/k/v projections for its 4 heads (column-parallel),
RoPE, causal attention, and a row-parallel partial o_proj; the host sums
the 4 partials per batch.

v2 (363us -> ~287us): fp8 (e4m3) DoubleRow matmuls at 2x bf16 PE rate
for the bulk of the work, exploiting the loose 2e-2 rel-err gate
(final rel err ~8.5e-3). Error analysis: softmax here is broad (logit
std ~= 1), so iid fp8 quantization noise on q/k/v/P averages down by
~1/sqrt(N_keys) for late queries; only EARLY queries (few keys) and
the final o_proj see fp8 noise unattenuated. Hence:
  - queries/keys 0..255 run a bf16/fp16 exact path (bf16 q/k/v
    projections, fp16 probabilities, fp16 V copy),
  - everything else uses fp8 DoubleRow for Q/K/V projections and P@V
    in pairs of adjacent key tiles (P stored fp8 with exp bias -2 so
    values fit e4m3's +-240 range - the bias cancels in softmax
    normalization),
  - scores stay bf16: DoubleRow needs the d=128 contraction split to
    K=64, and K=64 DR measures HALF the K=128 rate (417 vs 211 ns per
    512-col matmul) - zero gain, so don't (tried, reverted),
  - o_proj stays bf16 (no averaging after it; fp8 would be ~3.5% err).
Weights are host-prescaled by 32 (W elems ~ N(0, 1/2048) would land in
e4m3's subnormal range); the 1/32 is folded into the RoPE tables (bf16)
and the V-eviction scale.

Engine-placement lessons (measured, not guessed):
  - GpSimd DSP is ~3x slower than DVE per tensor op (1.15us vs ~0.35us
    for a [128,512] fp16 add) and swapping op kinds forces ~7us library
    reloads; it now does ONLY dma_start issue. (Moving rope-adds or
    smacc there cost 50-290us total - reverted.)
  - Softmax sums accumulate on the PE: per-pair ones8 [128,2,128]
    DoubleRow matmuls into a psum whose 128 identical rows make the
    sum pre-broadcast, so the normalize chain is just reciprocal (DVE,
    full height) -> aot multiply. No partition_broadcast at all.
    (DVE smacc adds convoy the mask->PV chain: +80us - reverted.)
  - fp8 stale-byte hazard: pt8 ring slots hold old fp16 bytes that
    alias to e4m3 NaN, and NaN*0=NaN, so diagonal-pair gaps are
    memset to 0 instead of relying on the mask multiply.
  - Phase P runs fp8-first: the first V matmuls gate on ~0.5MB of fp8
    stream instead of 2.6MB of bf16 under the ~20us DMA bandwidth
    ramp; bf16 streams (xc0, wv, wq/wk) land during fp8 compute.
  - Q and K are projected per head, head 0 first, so attention starts
    while later heads' RoPE evictions drain the DVE queue.
  - Do NOT emit chunk 2's o_proj inside chunk 3's head loop: +55us
    (tried, reverted; cause unclear - keep emission after the loop).

Carried over from v1: SBUF-resident per-head QT/KT, 4-deep weight tile
rings, staged per-head normalize chain (one stage pumped per tile of
the following head), o_proj deferred one chunk and its bf16 partials
spread over 3 DMA queues.
"""
import os
import sys

if "/opt/trn_rl_repo" not in sys.path:
    sys.path.insert(0, "/opt/trn_rl_repo")

import numpy as np
import ml_dtypes

import concourse.bass as bass
import concourse.mybir as mybir
import concourse.tile as tile
from concourse import bacc
from concourse.bass_utils import run_bass_kernel_spmd
from contextlib import ExitStack

P = 128
B, S, HID, NH = 2, 2048, 2048, 16
HD = HID // NH              # 128
H = 4                       # heads per core
DPC = H * HD                # 512 dims per core
KO = HID // P               # 16 contraction chunks
SC = S // 512               # 4 seq chunks of 512
ST = S // P                 # 16 seq tiles of 128
SCALE = 1.0 / float(np.sqrt(HD))
WS = 32.0                   # host weight prescale (fp8 range)
PB = -2.0                   # exp bias for fp8 P (cancels in softmax)

f32 = mybir.dt.float32
bf16 = mybir.dt.bfloat16
fp16 = mybir.dt.float16
fp8 = mybir.dt.float8e4
DR = mybir.MatmulPerfMode.DoubleRow

_CACHED_NC = None


def build_nc():
    DBG = os.environ.get("KDBG", "0") == "1"
    AF = mybir.ActivationFunctionType
    nc = bacc.Bacc(None, target_bir_lowering=False)

    # chunk 0 of x in bf16 (early-query exact path); chunks 1-3 fp8
    xt = nc.declare_dram_parameter("xt", [P, KO, 256], bf16, isOutput=False)
    xt8 = nc.declare_dram_parameter("xt8", [P, KO, 1792], fp8, isOutput=False)
    wq = nc.declare_dram_parameter("wq", [H, P, KO, HD], bf16, isOutput=False)
    wk = nc.declare_dram_parameter("wk", [H, P, KO, HD], bf16, isOutput=False)
    wq8 = nc.declare_dram_parameter("wq8", [H, P, KO, HD], fp8, isOutput=False)
    wk8 = nc.declare_dram_parameter("wk8", [H, P, KO, HD], fp8, isOutput=False)
    wv = nc.declare_dram_parameter("wv", [P, KO, DPC], bf16, isOutput=False)
    wv8 = nc.declare_dram_parameter("wv8", [P, KO, DPC], fp8, isOutput=False)
    wo = nc.declare_dram_parameter("wo", [P, H, HID], bf16, isOutput=False)
    cosf = nc.declare_dram_parameter("cosf", [P, S], bf16, isOutput=False)
    sinf = nc.declare_dram_parameter("sinf", [P, S], bf16, isOutput=False)
    bmask = nc.declare_dram_parameter("bmask", [P, H, 512], fp8, isOutput=False)
    # bf16 partials: host sums the 4 head-group partials in f32
    out_p = nc.declare_dram_parameter("out_p", [S, HID], bf16, isOutput=True)
    if DBG:
        dbg_qt = nc.declare_dram_parameter("dbg_qt", [P, S], bf16, isOutput=True)
        dbg_kt = nc.declare_dram_parameter("dbg_kt", [P, S], bf16, isOutput=True)
        dbg_vsb = nc.declare_dram_parameter("dbg_vsb", [P, ST, H, 128], fp8,
                                            isOutput=True)
        dbg_pt = nc.declare_dram_parameter("dbg_pt", [P, 2, 512], fp8,
                                           isOutput=True)
        dbg_sm = nc.declare_dram_parameter("dbg_sm", [1, 512], f32, isOutput=True)
        dbg_aot = nc.declare_dram_parameter("dbg_aot", [SC, P, H, 512], bf16,
                                            isOutput=True)

    out3 = out_p.rearrange("(st p) n -> p st n", p=P)

    with tile.TileContext(nc) as tc:
        with ExitStack() as top:
            vpool = top.enter_context(tc.tile_pool(name="vpool", bufs=1))
            qkres = top.enter_context(tc.tile_pool(name="qkres", bufs=1))
            const = top.enter_context(tc.tile_pool(name="const", bufs=1))

            # fp8 V for all 16 tiles (fp8 P@V, chunks 1-3) + fp16 copy of
            # tiles 0-3 for chunk 0's bf16 path
            vsb = vpool.tile([P, ST, H, 128], fp8)
            vsb0 = vpool.tile([P, 2, H, 128], fp16)
            qt_h = [qkres.tile([P, S], bf16, tag=f"qt{h}", name=f"qt{h}")
                    for h in range(H)]
            kt_h = [qkres.tile([P, S], bf16, tag=f"kt{h}", name=f"kt{h}")
                    for h in range(H)]

            zb = const.tile([P, 1], f32)
            nc.vector.memset(zb[:], 0.0)
            nb = const.tile([P, 1], f32)
            nc.vector.memset(nb[:], PB)
            # warm the scalar-engine exp table so the first attention tile
            # doesn't eat the ACT_TABLE_LOAD latency
            warm = const.tile([P, 1], fp16)
            nc.scalar.activation(warm[:], zb[:], AF.Exp, bias=zb[:], scale=1.0)
            bmt = const.tile([P, H, 512], fp8)

            # ---------------- Phase P: projections ----------------
            # fp8 work leads (first matmul gated on ~0.5MB of fp8 stream vs
            # 2.6MB of bf16 under the DMA bandwidth ramp); the bf16 streams
            # land while the fp8 projections compute.
            with ExitStack() as ctx:
                xpool = ctx.enter_context(tc.tile_pool(name="xp", bufs=1))
                wvpool = ctx.enter_context(tc.tile_pool(name="wvp", bufs=1))
                pp = ctx.enter_context(tc.tile_pool(name="pp", bufs=4, space="PSUM"))

                wvq = [wvpool.tile([P, KO // 4, DPC], bf16, tag=f"wv{j}",
                                   name=f"wv{j}") for j in range(4)]
                # fp8 streams: x chunk 1 in quarter tiles (fine-grained DMA
                # watermarks under the ramp), chunks 2-3 whole; wv8 quarters
                # (pair-aligned)
                x8a = xpool.tile([P, KO, 256], fp8, tag="x8a", name="x8a")
                xq1 = [xpool.tile([P, KO // 4, 512], fp8, tag=f"xq1{j}",
                                  name=f"xq1{j}") for j in range(4)]
                xs8 = [None, xpool.tile([P, KO, 512], fp8, tag="x82", name="x82"),
                       xpool.tile([P, KO, 512], fp8, tag="x83", name="x83")]
                wv8q = [wvpool.tile([P, KO // 4, DPC], fp8, tag=f"w8{j}",
                                    name=f"w8{j}") for j in range(4)]
                cspool = ctx.enter_context(tc.tile_pool(name="cs", bufs=1))
                rtmp = ctx.enter_context(tc.tile_pool(name="rt", bufs=3))
                wpool = ctx.enter_context(tc.tile_pool(name="wqk", bufs=4))
                w8pool = ctx.enter_context(tc.tile_pool(name="wqk8", bufs=4))
                cosT = cspool.tile([P, S], bf16)
                sinT = cspool.tile([P, S], bf16)
                xc0 = xpool.tile([P, KO, 256], bf16, tag="xc0", name="xc0")

                # fp8 moving/stationary slice helpers: kp indexes ko-pairs
                def xq1_pair(kp, so=None):
                    t = xq1[kp // 2][:, 2 * (kp % 2):2 * (kp % 2) + 2]
                    return t if so is None else t[:, :, so:so + P]

                def x8_pair(sc, kp, so=None):
                    if sc == 0:
                        t = x8a[:, 2 * kp:2 * kp + 2]
                    elif sc == 1:
                        return xq1_pair(kp, so)
                    else:
                        t = xs8[sc - 1][:, 2 * kp:2 * kp + 2]
                    return t if so is None else t[:, :, so:so + P]

                # critical fp8 set over all three queues
                nc.sync.dma_start(wv8q[0][:], wv8[:, 0:4])
                nc.scalar.dma_start(xq1[0][:], xt8[:, 0:4, 256:768])
                nc.gpsimd.dma_start(wv8q[1][:], wv8[:, 4:8])
                nc.sync.dma_start(xq1[1][:], xt8[:, 4:8, 256:768])
                nc.scalar.dma_start(wv8q[2][:], wv8[:, 8:12])
                nc.gpsimd.dma_start(xq1[2][:], xt8[:, 8:12, 256:768])
                nc.sync.dma_start(wv8q[3][:], wv8[:, 12:16])
                nc.scalar.dma_start(xq1[3][:], xt8[:, 12:16, 256:768])
                nc.gpsimd.dma_start(x8a[:, 0:8], xt8[:, 0:8, 0:256])
                nc.sync.dma_start(x8a[:, 8:16], xt8[:, 8:16, 0:256])

                # V tiles: fp8 DoubleRow (x stationary pair, wv moving)
                def v_block8(sc, sts=None):
                    for st in (sts if sts is not None else range(sc * 4, sc * 4 + 4)):
                        so = (st % 4) * P if sc != 0 else (st - 2) * P
                        ps = pp.tile([P, 512], f32, tag="vproj", name="ps")
                        for kp in range(KO // 2):
                            nc.tensor.matmul(
                                ps[:],
                                x8_pair(sc, kp, so),
                                wv8q[kp // 2][:, 2 * (kp % 2):2 * (kp % 2) + 2],
                                start=(kp == 0),
                                stop=(kp == KO // 2 - 1),
                                perf_mode=DR,
                            )
                        nc.vector.tensor_scalar_mul(
                            vsb[:, st].rearrange("p h d -> p (h d)"),
                            ps[:], 1.0 / WS,
                        )

                v_block8(1)
                v_block8(0, sts=(2, 3))
                nc.sync.dma_start(xs8[1][:, 0:8], xt8[:, 0:8, 768:1280])
                nc.scalar.dma_start(xs8[1][:, 8:16], xt8[:, 8:16, 768:1280])
                nc.gpsimd.dma_start(cosT[:], cosf[:])
                v_block8(2)
                nc.sync.dma_start(xs8[2][:, 0:8], xt8[:, 0:8, 1280:1792])
                nc.scalar.dma_start(xs8[2][:, 8:16], xt8[:, 8:16, 1280:1792])
                nc.gpsimd.dma_start(sinT[:], sinf[:])
                # bf16 streams for the early-query exact path
                nc.sync.dma_start(wvq[0][:], wv[:, 0:4])
                nc.scalar.dma_start(wvq[1][:], wv[:, 4:8])
                nc.gpsimd.dma_start(xc0[:, 0:8], xt[:, 0:8, :])

                v_block8(3)
                nc.sync.dma_start(wvq[2][:], wv[:, 8:12])
                nc.scalar.dma_start(xc0[:, 8:16], xt[:, 8:16, :])
                nc.gpsimd.dma_start(wvq[3][:], wv[:, 12:16])
                nc.gpsimd.dma_start(bmt[:], bmask[:])

                # V tiles 0-1: bf16 from x cols 0:256
                for st in range(2):
                    so = st * P
                    ps = pp.tile([P, 512], f32, tag="vproj", name="ps")
                    for ko in range(KO):
                        nc.tensor.matmul(
                            ps[:],
                            xc0[:, ko, so:so + P],
                            wvq[ko // 4][:, ko % 4],
                            start=(ko == 0),
                            stop=(ko == KO - 1),
                        )
                    # chunk-0 V: fp16 copy (true scale) + fp8 copy, both DVE
                    nc.vector.tensor_scalar_mul(
                        vsb0[:, st].rearrange("p h d -> p (h d)"),
                        ps[:], 1.0 / WS,
                    )
                    nc.vector.tensor_scalar_mul(
                        vsb[:, st].rearrange("p h d -> p (h d)"),
                        ps[:], 1.0 / WS,
                    )

                def rope_evict(ps, dst, ssl, W=512):
                    # RoPE eviction. The rotate's partition swap runs on the
                    # scalar engine (idle during phase P) as two shifted
                    # PSUM->fp16 copies, so the DVE does one full-height fp16
                    # mul instead of two half-height f32-source muls; DVE
                    # per-chunk time drops below the PE's, which was the QK
                    # phase pacer.
                    sw = rtmp.tile([P, 512], fp16, tag="sw")
                    t0 = rtmp.tile([P, 512], fp16, tag="t0")
                    t1 = rtmp.tile([P, 512], fp16, tag="t1")
                    nc.scalar.activation(sw[0:64, 0:W], ps[64:128, 0:W],
                                         AF.Copy)
                    nc.scalar.activation(sw[64:128, 0:W], ps[0:64, 0:W],
                                         AF.Copy)
                    nc.vector.tensor_mul(t0[:, 0:W], sw[:, 0:W], sinT[:, ssl])
                    nc.vector.tensor_mul(t1[:, 0:W], ps[:, 0:W], cosT[:, ssl])
                    nc.vector.tensor_add(dst[:, ssl], t1[:, 0:W], t0[:, 0:W])

                # Q and K per head, head 0 first: attention (c,h) gates on
                # the per-head QT/KT tile's LAST write, so finishing whole
                # heads early lets the attention stream start while later
                # heads' RoPE evictions drain through the DVE queue.
                for h in range(H):
                    for w4, w84, dst_h in ((wq, wq8, qt_h), (wk, wk8, kt_h)):
                        w8t = w8pool.tile([P, KO, HD], fp8, tag="w8")
                        nc.gpsimd.dma_start(w8t[:], w84[h])
                        wt = wpool.tile([P, KO, HD], bf16, tag="w")
                        nc.scalar.dma_start(wt[:], w4[h])
                        # chunks 1-3 + cols 256:512 of chunk 0: fp8 DoubleRow
                        for sc in range(1, SC):
                            ssl = slice(sc * 512, (sc + 1) * 512)
                            ps = pp.tile([P, 512], f32, tag="proj")
                            for kp in range(KO // 2):
                                nc.tensor.matmul(
                                    ps[:],
                                    w8t[:, 2 * kp:2 * kp + 2],
                                    x8_pair(sc, kp),
                                    start=(kp == 0),
                                    stop=(kp == KO // 2 - 1),
                                    perf_mode=DR,
                                )
                            rope_evict(ps, dst_h[h], ssl)
                        ps = pp.tile([P, 512], f32, tag="proj")
                        for kp in range(KO // 2):
                            nc.tensor.matmul(
                                ps[:, 0:256],
                                w8t[:, 2 * kp:2 * kp + 2],
                                x8_pair(0, kp),
                                start=(kp == 0),
                                stop=(kp == KO // 2 - 1),
                                perf_mode=DR,
                            )
                        rope_evict(ps, dst_h[h], slice(256, 512), W=256)
                        # cols 0:256: bf16 exact
                        ps = pp.tile([P, 512], f32, tag="proj")
                        for ko in range(KO):
                            nc.tensor.matmul(
                                ps[:, 0:256],
                                wt[:, ko],
                                xc0[:, ko],
                                start=(ko == 0),
                                stop=(ko == KO - 1),
                            )
                        rope_evict(ps, dst_h[h], slice(0, 256), W=256)

            if DBG:
                nc.sync.dma_start(dbg_qt[:], qt_h[0][:])
                nc.sync.dma_start(dbg_kt[:], kt_h[0][:])
                nc.sync.dma_start(dbg_vsb[:], vsb[:])

            # ------------- Phase A: attention + interleaved o_proj -------------
            with ExitStack() as ctx:
                ppool = ctx.enter_context(tc.tile_pool(name="ppool", bufs=6))
                smpool = ctx.enter_context(tc.tile_pool(name="smp", bufs=2))
                stage = ctx.enter_context(tc.tile_pool(name="stage", bufs=4))
                aopool = ctx.enter_context(tc.tile_pool(name="ao", bufs=1))
                wopool = ctx.enter_context(tc.tile_pool(name="wop", bufs=1))
                ost = ctx.enter_context(tc.tile_pool(name="ost", bufs=4))
                # attention psum pools live in an inner scope released
                # before the FINAL chunk's o_proj, whose eviction ring can
                # then go 4 deep (ring-2 makes it eviction-paced)
                opo = ctx.enter_context(tc.tile_pool(name="opo", bufs=2, space="PSUM"))
                pctx = ExitStack()
                spsum = pctx.enter_context(tc.tile_pool(name="sps", bufs=2, space="PSUM"))
                opsum = pctx.enter_context(tc.tile_pool(name="ops", bufs=2, space="PSUM"))
                smq = pctx.enter_context(tc.tile_pool(name="smq", bufs=2, space="PSUM"))

                # full-width ones: the softmax-sum matmuls write the sum
                # broadcast across all 128 psum partitions, so the normalize
                # chain needs no gpsimd partition_broadcast (slow DSP, and
                # mixing op kinds forces ~7us library swaps)
                ones_col = const.tile([P, P], fp16)
                nc.vector.memset(ones_col[:], 1.0)
                ones8 = const.tile([P, 2, P], fp8)
                nc.vector.memset(ones8[:], 1.0)
                # wot's dma is issued after chunk 0's first head (it would
                # gate c0's first matmuls via the queue watermark otherwise)
                wot = wopool.tile([P, H, HID], bf16)

                aot_c = [
                    aopool.tile([P, H, 512], bf16, tag=f"aot{c}", name=f"aot{c}")
                    for c in range(SC)
                ]

                def emit_oproj(cc, pool=None, tag="po"):
                    for st4 in range(4):
                        st = cc * 4 + st4
                        for nch in range(4):
                            g = st4 * 4 + nch
                            pso = (pool or opo).tile([P, 512], f32, tag=tag,
                                                     name="pso")
                            for dc in range(H):
                                nc.tensor.matmul(
                                    pso[:],
                                    aot_c[cc][:, dc, st4 * P:(st4 + 1) * P],
                                    wot[:, dc, nch * 512:(nch + 1) * 512],
                                    start=(dc == 0),
                                    stop=(dc == H - 1),
                                )
                            # PSUM->SBUF eviction split between scalar ACT
                            # and DVE (gpsimd cannot read PSUM); bf16 out
                            # halves the write stream, spread over 3 queues
                            ob = ost.tile([P, 512], bf16, tag="ob", name="ob")
                            if g % 2 == 0:
                                nc.scalar.activation(ob[:], pso[:], AF.Copy)
                            else:
                                nc.vector.tensor_copy(ob[:], pso[:])
                            eng = (nc.sync, nc.gpsimd, nc.scalar)[g % 3]
                            eng.dma_start(
                                out3[:, st, nch * 512:(nch + 1) * 512], ob[:]
                            )

                # Per-head normalize chain (sm matmul -> rcp -> gpsimd
                # broadcast -> DVE mul), pumped ONE STAGE PER TILE of the
                # following head(s). Emitting the whole chain at once parks
                # ops at the head of the in-order DVE/tensor queues waiting
                # on cross-engine inputs and convoys the tile stream; staged,
                # every op is data-ready when its queue reaches it.
                pending = []

                def norm_pump():
                    if not pending:
                        return
                    e = pending[0]
                    s = e["s"]
                    e["s"] += 1
                    w = e["w"]
                    if s == 0:
                        # fp8 chunks accumulate the softmax sum on the PE
                        # (ones8 DoubleRow per pair) so e["smps"] is already
                        # set; bf16 parts reduce the DVE-accumulated smacc
                        if e["smps"] is None:
                            e["smps"] = smq.tile([P, 512], f32, tag="smp",
                                                 name="smp")
                            nc.tensor.matmul(
                                e["smps"][:, 0:w], ones_col[:],
                                e["sm"][:, 0:w],
                                start=True, stop=True,
                            )
                    elif s == 1:
                        e["rcp"] = stage.tile([P, 512], f32, tag="rcp",
                                              name="rcp")
                        nc.vector.reciprocal_approx_fast(
                            e["rcp"][:, 0:w], e["smps"][:, 0:w])
                    else:
                        qo = e["qo"]
                        nc.vector.tensor_mul(
                            aot_c[e["c"]][:, e["h"], qo:qo + w],
                            e["ob"][:, 0:w], e["rcp"][:, 0:w])
                        pending.pop(0)

                for c in range(SC):
                    qsl = lambda off: slice(c * 512 + off, (c + 1) * 512)
                    nt = 4 * (c + 1)
                    for h in range(H):
                        # finish the chain that owns the recycled ring slot
                        # before reallocating it (only bites in c0's short
                        # 4-tile heads)
                        while len(pending) >= 2:
                            norm_pump()
                        # attn_outT accumulator [d, sq]
                        ob_ps = opsum.tile([P, 512], f32, tag="obp", name="obp")
                        if c == 0:
                            # chunk 0 part A (queries 0:255): bf16/fp16 exact
                            # path (early-query accuracy: fp8 noise doesn't
                            # average over few keys)
                            smacc = smpool.tile([P, 512], fp16, tag="sma",
                                                name="sma")
                            for ti, t in enumerate(range(2)):
                                norm_pump()
                                off = P * t
                                ps = spsum.tile([P, 512], f32, tag="s")
                                nc.tensor.matmul(
                                    ps[:, off:256],
                                    kt_h[h][:, t * P:(t + 1) * P],
                                    qt_h[h][:, off:256],
                                    start=True,
                                    stop=True,
                                )
                                pt = ppool.tile([P, 512], fp16, tag="pt")
                                nc.scalar.activation(
                                    pt[:, off:256], ps[:, off:256], AF.Exp,
                                    bias=zb[:], scale=SCALE,
                                )
                                nc.vector.tensor_mul(
                                    pt[:, off:256], pt[:, off:256],
                                    bmt[:, t, off:256]
                                )
                                nc.tensor.matmul(
                                    ob_ps[:, off:256],
                                    vsb0[:, t, h],
                                    pt[:, off:256],
                                    start=(ti == 0),
                                    stop=(ti == 1),
                                )
                                if ti == 0:
                                    nc.vector.tensor_copy(
                                        smacc[:, 0:256], pt[:, 0:256])
                                else:
                                    nc.vector.tensor_add(
                                        smacc[:, off:256], smacc[:, off:256],
                                        pt[:, off:256],
                                    )
                            pending.append(
                                {"s": 0, "c": 0, "h": h, "ob": ob_ps,
                                 "sm": smacc, "smps": None, "w": 256,
                                 "qo": 0}
                            )
                            # part B (queries 256:511): fp8 pairs over key
                            # tiles (2,3) diagonal then (0,1)
                            while len(pending) >= 2:
                                norm_pump()
                            ob_ps = opsum.tile([P, 512], f32, tag="obp",
                                               name="obp")
                            smq_ps = smq.tile([P, 512], f32, tag="smp",
                                              name="smq_ps")
                            for pi, (ta, tb) in enumerate(((2, 3), (0, 1))):
                                pt8 = ppool.tile([P, 2, 512], fp8, tag="pt")
                                offs = []
                                for i, t in enumerate((ta, tb)):
                                    norm_pump()
                                    r = t - 2
                                    off = P * max(r, 0)
                                    offs.append(off)
                                    ps = spsum.tile([P, 512], f32, tag="s")
                                    nc.tensor.matmul(
                                        ps[:, off:256],
                                        kt_h[h][:, t * P:(t + 1) * P],
                                        qt_h[h][:, 256 + off:512],
                                        start=True,
                                        stop=True,
                                    )
                                    nc.scalar.activation(
                                        pt8[:, i, off:256], ps[:, off:256],
                                        AF.Exp, bias=nb[:], scale=SCALE,
                                    )
                                    if r >= 0:
                                        if off > 0:
                                            nc.vector.memset(
                                                pt8[:, i, 0:off], 0.0)
                                        nc.vector.tensor_mul(
                                            pt8[:, i, off:256],
                                            pt8[:, i, off:256],
                                            bmt[:, r, off:256],
                                        )
                                nc.tensor.matmul(
                                    ob_ps[:, 0:256],
                                    vsb[:, ta:ta + 2, h],
                                    pt8[:, :, 0:256],
                                    start=(pi == 0),
                                    stop=(pi == 1),
                                    perf_mode=DR,
                                )
                                nc.tensor.matmul(
                                    smq_ps[:, 0:256],
                                    ones8[:],
                                    pt8[:, :, 0:256],
                                    start=(pi == 0),
                                    stop=(pi == 1),
                                    perf_mode=DR,
                                )
                            pending.append(
                                {"s": 0, "c": 0, "h": h, "ob": ob_ps,
                                 "sm": None, "smps": smq_ps, "w": 256,
                                 "qo": 256}
                            )
                        else:
                            # chunks 1-3: fp8 P@V in DoubleRow pairs of
                            # adjacent key tiles; probabilities exp(s-2) in
                            # e4m3, causal mask on DVE, softmax sums via
                            # ones8 DoubleRow into a [16,512] psum (row 0).
                            # diagonal pairs first: their exp+mask latency
                            # hides behind the dense unmasked tail of this
                            # head and the previous head's stream
                            smq_ps = smq.tile([P, 512], f32, tag="smp",
                                              name="smq_ps")
                            t_order = list(range(4 * c, nt)) + list(range(0, 4 * c))
                            npair = nt // 2
                            for pi in range(npair):
                                ta, tb = t_order[2 * pi], t_order[2 * pi + 1]
                                pt8 = ppool.tile([P, 2, 512], fp8, tag="pt")
                                offs = []
                                for i, t in enumerate((ta, tb)):
                                    norm_pump()
                                    if c == SC - 1 and h >= 2:
                                        norm_pump()
                                    r = t - 4 * c
                                    off = P * max(r, 0)
                                    offs.append(off)
                                    ps = spsum.tile([P, 512], f32, tag="s")
                                    nc.tensor.matmul(
                                        ps[:, off:512],
                                        kt_h[h][:, t * P:(t + 1) * P],
                                        qt_h[h][:, qsl(off)],
                                        start=True,
                                        stop=True,
                                    )
                                    nc.scalar.activation(
                                        pt8[:, i, off:512], ps[:, off:512],
                                        AF.Exp, bias=nb[:], scale=SCALE,
                                    )
                                    if r >= 0:
                                        # slot 1 of a diagonal pair: the PV
                                        # matmul reads from the pair's base
                                        # offset, so zero the stale region
                                        # below this slot's diagonal (mask
                                        # multiply can't: stale fp8 bytes can
                                        # be NaN encodings and NaN*0=NaN)
                                        offp = P * 2 * pi
                                        if off > offp:
                                            nc.vector.memset(
                                                pt8[:, i, offp:off], 0.0)
                                        nc.vector.tensor_mul(
                                            pt8[:, i, off:512],
                                            pt8[:, i, off:512],
                                            bmt[:, r, off:512],
                                        )
                                off0 = min(offs)
                                nc.tensor.matmul(
                                    ob_ps[:, off0:512],
                                    vsb[:, ta:ta + 2, h],
                                    pt8[:, :, off0:512],
                                    start=(pi == 0),
                                    stop=(pi == npair - 1),
                                    perf_mode=DR,
                                )
                                nc.tensor.matmul(
                                    smq_ps[:, off0:512],
                                    ones8[:],
                                    pt8[:, :, off0:512],
                                    start=(pi == 0),
                                    stop=(pi == npair - 1),
                                    perf_mode=DR,
                                )
                                if DBG and c == 1 and h == 0 and pi == 0:
                                    nc.sync.dma_start(dbg_pt[:], pt8[:])
                            if DBG and c == 1 and h == 0:
                                sdump = stage.tile([1, 512], f32, tag="sdmp",
                                                   name="sdump")
                                nc.vector.tensor_copy(sdump[:], smq_ps[0:1])
                                nc.sync.dma_start(dbg_sm[:], sdump[:])
                        if c > 0:
                            pending.append(
                                {"s": 0, "c": c, "h": h, "ob": ob_ps,
                                 "sm": None, "smps": smq_ps, "w": 512,
                                 "qo": 0}
                            )
                        if c == 0 and h == 0:
                            # issue late so it doesn't gate c0's matmuls
                            nc.gpsimd.dma_start(wot[:], wo[:])

                    # o_proj deferred by one chunk: its aot inputs are then
                    # guaranteed ready, so the PE stream never stalls on the
                    # normalize tail
                    if c > 0:
                        emit_oproj(c - 1)
                while pending:
                    norm_pump()
                pctx.close()
                opof = ctx.enter_context(
                    tc.tile_pool(name="opof", bufs=6, space="PSUM"))
                emit_oproj(SC - 1, pool=opof, tag="pof")
                if DBG:
                    for c in range(SC):
                        nc.sync.dma_start(dbg_aot[c], aot_c[c][:])

    nc.compile()
    return nc


def _host_prep(hidden_states, position_ids, Wq, Wk, Wv, Wo):
    """Build the 8 per-core input maps (bf16/fp8 weights/activations)."""
    inv_freq = 1.0 / (10000.0 ** (np.arange(0, HD, 2, dtype=np.float32) / HD))
    t = np.arange(S, dtype=np.float32)
    freqs = np.outer(t, inv_freq).astype(np.float32)  # [S, 64]

    bm = np.empty((P, H, 512), dtype=np.float32)
    i = np.arange(P)[:, None, None]
    r = np.arange(H)[None, :, None]
    j = np.arange(512)[None, None, :]
    bm[:] = np.where(i + P * r <= j, 1.0, 0.0)
    bm = bm.astype(ml_dtypes.float8_e4m3)

    in_maps = []
    per_batch = []
    for b in range(B):
        xT = np.ascontiguousarray(hidden_states[b].T)  # [HID, S]
        xt_sw = np.ascontiguousarray(
            xT.reshape(KO, P, S).transpose(1, 0, 2)
        )  # [P, KO, S] f32
        xt_b = np.ascontiguousarray(xt_sw[:, :, 0:256]).astype(
            ml_dtypes.bfloat16)
        xt_8 = np.ascontiguousarray(xt_sw[:, :, 256:2048]).astype(
            ml_dtypes.float8_e4m3)
        fp = freqs[position_ids[b]]  # [S, 64]
        ch = np.cos(fp).T / WS       # [64, S]; 1/32 weight descale folded in
        sh = np.sin(fp).T / WS
        cosf = np.ascontiguousarray(np.concatenate([ch, ch], axis=0)).astype(
            ml_dtypes.bfloat16)   # [128, S]
        sinf = np.ascontiguousarray(np.concatenate([-sh, sh], axis=0)).astype(
            ml_dtypes.bfloat16)  # signed
        per_batch.append((xt_b, xt_8, cosf, sinf))

    for core in range(8):
        b, hg = core // 4, core % 4
        sl = slice(hg * DPC, (hg + 1) * DPC)
        xt_b, xt_8, cosf, sinf = per_batch[b]
        wq_sw = np.ascontiguousarray(
            Wq[sl].T.reshape(KO, P, H, HD).transpose(2, 1, 0, 3)) * WS
        wk_sw = np.ascontiguousarray(
            Wk[sl].T.reshape(KO, P, H, HD).transpose(2, 1, 0, 3)) * WS
        wv_sw = np.ascontiguousarray(
            Wv[sl].T.reshape(KO, P, DPC).transpose(1, 0, 2)) * WS
        wo_sw = np.ascontiguousarray(
            Wo[:, sl].T.reshape(H, HD, HID).transpose(1, 0, 2)
        ).astype(ml_dtypes.bfloat16)  # [P, H, HID]
        in_maps.append({
            "xt": xt_b, "xt8": xt_8,
            "wq": wq_sw.astype(ml_dtypes.bfloat16),
            "wk": wk_sw.astype(ml_dtypes.bfloat16),
            "wq8": wq_sw.astype(ml_dtypes.float8_e4m3),
            "wk8": wk_sw.astype(ml_dtypes.float8_e4m3),
            "wv": wv_sw.astype(ml_dtypes.bfloat16),
            "wv8": wv_sw.astype(ml_dtypes.float8_e4m3),
            "wo": wo_sw,
            "cosf": cosf, "sinf": sinf, "bmask": bm,
        })
    return in_maps


def kernel(hidden_states, attention_mask, position_ids, Wq, Wk, Wv, Wo,
           _trace=False, _trace_kwargs=None):
    global _CACHED_NC
    hidden_states = np.asarray(hidden_states, dtype=np.float32)
    position_ids = np.asarray(position_ids)
    Wq, Wk, Wv, Wo = (np.asarray(w, dtype=np.float32) for w in (Wq, Wk, Wv, Wo))

    if _CACHED_NC is None:
        _CACHED_NC = build_nc()
    nc = _CACHED_NC

    in_maps = _host_prep(hidden_states, position_ids, Wq, Wk, Wv, Wo)
    res = run_bass_kernel_spmd(
        nc, in_maps, list(range(8)), trace=_trace, **(_trace_kwargs or {})
    )

    out = np.empty((B, S, HID), dtype=np.float32)
    for b in range(B):
        acc = res.results[b * 4]["out_p"].astype(np.float32)
        for hg in range(1, 4):
            acc = acc + res.results[b * 4 + hg]["out_p"].astype(np.float32)
        out[b] = acc
    if _trace:
        return out, res
    return out
